# revision 67
# baseline (speedup 1.0000x reference)
"""Trainium2 Bass kernel for nn_MAMoE (conv-MoE -> row attention -> MLP-MoE).

Sharding: 8 cores = (batch b in 0..3) x (H-half in 0..1). All routing is
per-token; the reference's swapaxes(1,2) means attention row r produces
output column w=r, so each core independently computes the full pipeline
for its 48 attention rows and the host reassembles along W.

v3: all matmuls bf16 (f32 PSUM accumulate); per-branch minimal conv
padding (100/96/112 row pitch); two-stage attention emission so the
softmax chain hides under the next conv group's matmuls; software-
pipelined phase B (ups run 2 iterations ahead of downs); first tile's
gating hoisted into the phase A tail.
"""
import contextlib

import numpy as np
import ml_dtypes

import concourse.bass as bass
import concourse.bass_isa as bass_isa
import concourse.mybir as mybir
import concourse.tile as tile
from concourse import bacc
from concourse.bass_utils import run_bass_kernel_spmd
from concourse.masks import make_identity

F32 = mybir.dt.float32
BF16 = mybir.dt.bfloat16
BFNP = ml_dtypes.bfloat16

B, HH, WW, C = 4, 96, 96, 384
HD = 128
SCALE = float((HD // 3) ** -0.5)  # 42**-0.5
N_CORES = 8
R = 48            # attention rows per core
T = R * 96        # tokens per core = 4608
NT = 512          # tokens per MLP tile
NTILES = T // NT  # 9
GN = 4 * 96       # tokens per attention group = 384

# per-branch padded-plane geometry
GEOM = [
    dict(SP=100, NR=52, OFF=8, BUF=5248, pad_r=2, pad_c=2),   # 3x3 convs
    dict(SP=96, NR=64, OFF=0, BUF=6144, pad_r=8, pad_c=0),    # (9,1) convs
    dict(SP=104, NR=48, OFF=8, BUF=5120, pad_r=0, pad_c=0),   # (1,9) convs
]
for _g in GEOM:
    _g["lo"] = _g["OFF"] + _g["pad_r"] * _g["SP"]
    _g["rlen"] = 48 * _g["SP"]
GEOM[2]["rlen"] = 47 * 104 + 96

TAPS_A = [
    [(dr, ds) for dr in (-1, 0, 1) for ds in (-1, 0, 1)],
    [(dr, 0) for dr in range(-4, 5)],
    [(0, ds) for ds in range(-4, 5)],
]
TAPS_B = [
    [(dr, ds) for dr in (-2, 0, 2) for ds in (-2, 0, 2)],
    [(dr, 0) for dr in range(-8, 9, 2)],
    [(0, ds) for ds in range(-8, 9, 2)],
]


def _row_off(i, r):
    g = GEOM[i]
    return g["OFF"] + (g["pad_r"] + r) * g["SP"] + g["pad_c"]


def _groups(i):
    g = GEOM[i]
    out = []
    fo = g["lo"]
    end = g["lo"] + g["rlen"]
    while fo < end:
        out.append((fo, min(NT, end - fo)))
        fo += NT
    return out


_CACHED_NC = None


def build_kernel():
    nc = bacc.Bacc("TRN2", target_bir_lowering=False, debug=False)

    xps = [nc.dram_tensor(f"xp{i}", [HD, GEOM[i]["BUF"]], BF16,
                          kind="ExternalInput").ap() for i in range(3)]
    wca = nc.dram_tensor("wca", [3, HD, 9, HD], BF16, kind="ExternalInput").ap()
    wcb = nc.dram_tensor("wcb", [3, HD, 9, HD], BF16, kind="ExternalInput").ap()
    bca = nc.dram_tensor("bca", [HD, 3], F32, kind="ExternalInput").ap()
    bcb = nc.dram_tensor("bcb", [HD, 3], F32, kind="ExternalInput").ap()
    wgd = nc.dram_tensor("wgd", [3, HD, HD], BF16, kind="ExternalInput").ap()
    eb3 = nc.dram_tensor("eb3", [3, 384], BF16, kind="ExternalInput").ap()
    wu = nc.dram_tensor("wu", [3, HD, HD], BF16, kind="ExternalInput").ap()
    wv = nc.dram_tensor("wv", [3, HD, HD], BF16, kind="ExternalInput").ap()
    lgb = nc.dram_tensor("lgb", [3, 4], F32, kind="ExternalInput").ap()
    wgf = nc.dram_tensor("wgf", [3, HD, HD], BF16, kind="ExternalInput").ap()
    w1 = nc.dram_tensor("w1", [3, HD, 3, 1536], BF16, kind="ExternalInput").ap()
    b1 = nc.dram_tensor("b1", [HD, 3, 12], F32, kind="ExternalInput").ap()
    w2 = nc.dram_tensor("w2", [3, HD, 12, C], BF16, kind="ExternalInput").ap()
    b2r = nc.dram_tensor("b2r", [3, C], BF16, kind="ExternalInput").ap()
    bpr = nc.dram_tensor("bpr", [HD, 3], F32, kind="ExternalInput").ap()
    out_cm = nc.dram_tensor("out_cm", [C, T], F32, kind="ExternalOutput").ap()

    with tile.TileContext(nc) as tc:
        with tc.tile_pool(name="persist", bufs=1) as persist, \
             tc.tile_pool(name="wpoolB", bufs=1) as wpoolB, \
             tc.tile_pool(name="gbpool", bufs=2) as gbpool, \
             tc.tile_pool(name="gpoolB", bufs=2) as gpoolB:
            stackA = contextlib.ExitStack()
            mpool = stackA.enter_context(tc.tile_pool(name="mpool", bufs=2))
            wpoolA = stackA.enter_context(tc.tile_pool(name="wpoolA", bufs=2))
            apool = stackA.enter_context(tc.tile_pool(name="apool", bufs=5))
            anorm = stackA.enter_context(tc.tile_pool(name="anorm", bufs=2))
            stackX = contextlib.ExitStack()
            xpool = stackX.enter_context(tc.tile_pool(name="xpool", bufs=2))
            # ---- branch-0 critical DMAs first (minimize first-matmul wait);
            # spread the first transfers across 4 queues so their fixed DMA
            # latencies overlap.
            xp_sb0 = xpool.tile([HD, 6144], BF16, tag="xp")
            c1, c2 = 1024, 3072
            nc.sync.dma_start(out=xp_sb0[:, :c1], in_=xps[0][:, :c1])
            wgd_sb0 = wpoolA.tile([HD, HD], BF16, tag="wgd")
            nc.scalar.dma_start(out=wgd_sb0, in_=wgd[0])
            wca_sb0 = wpoolA.tile([HD, 9, HD], BF16, tag="wca")
            nc.gpsimd.dma_start(out=wca_sb0, in_=wca[0])
            wcb_sb0 = wpoolA.tile([HD, 9, HD], BF16, tag="wcb")
            nc.gpsimd.dma_start(out=wcb_sb0, in_=wcb[0])
            wu_sb0 = wpoolA.tile([HD, HD], BF16, tag="wu")
            nc.gpsimd.dma_start(out=wu_sb0, in_=wu[0])
            wv_sb0 = wpoolA.tile([HD, HD], BF16, tag="wv")
            nc.gpsimd.dma_start(out=wv_sb0, in_=wv[0])
            nc.gpsimd.dma_start(out=xp_sb0[:, c1:c2], in_=xps[0][:, c1:c2])
            nc.sync.dma_start(out=xp_sb0[:, c2:GEOM[0]["BUF"]],
                              in_=xps[0][:, c2:])
            bca_sb = persist.tile([HD, 3], F32)
            bcb_sb = persist.tile([HD, 3], F32)
            lgb_sb = persist.tile([3, 4], F32)
            nc.sync.dma_start(out=bca_sb, in_=bca)
            nc.sync.dma_start(out=bcb_sb, in_=bcb)
            nc.sync.dma_start(out=lgb_sb, in_=lgb)

            br0 = dict(xp=xp_sb0, wgd=wgd_sb0, wca=wca_sb0, wcb=wcb_sb0,
                       wu=wu_sb0, wv=wv_sb0)

            identb = persist.tile([HD, HD], BF16)
            make_identity(nc, identb)

            xc_t = [persist.tile([HD, T], BF16, tag=f"xc{i}", name=f"xc{i}")
                    for i in range(3)]

            wB = {}

            def emit_phaseB_weight_dmas():
                wB["b1"] = wpoolB.tile([HD, 3, 12], F32, tag="b1", name="b1s")
                nc.sync.dma_start(out=wB["b1"], in_=b1)
                wB["b2r"] = wpoolB.tile([3, C], BF16, tag="b2r", name="b2rs")
                nc.sync.dma_start(out=wB["b2r"], in_=b2r)
                wB["wgf"] = wpoolB.tile([HD, 3, HD], BF16, tag="wgf", name="wgfs")
                nc.sync.dma_start(out=wB["wgf"],
                                  in_=wgf.rearrange("a p b -> p a b"))
                wB["bpr"] = wpoolB.tile([HD, 3], F32, tag="bpr", name="bprs")
                nc.sync.dma_start(out=wB["bpr"], in_=bpr)
                wB["eb3"] = wpoolB.tile([3, 384], BF16, tag="eb3", name="eb3s")
                nc.sync.dma_start(out=wB["eb3"], in_=eb3)
                wB["w1"] = []
                wB["w2"] = []
                for e in range(3):
                    t1 = wpoolB.tile([HD, 3, 1536], BF16, tag=f"w1_{e}",
                                     name=f"w1_{e}")
                    nc.sync.dma_start(out=t1, in_=w1[e])
                    wB["w1"].append(t1)
                    t2 = wpoolB.tile([HD, 12, C], BF16, tag=f"w2_{e}",
                                     name=f"w2_{e}")
                    nc.sync.dma_start(out=t2, in_=w2[e])
                    wB["w2"].append(t2)

            def gating_part1a(t, pool):
                """logits matmul for tile t."""
                t0 = t * NT
                plg = pool.tile([HD, NT], F32, tag="ps", name="plg")
                for kc in range(3):
                    nc.tensor.matmul(plg, wB["wgf"][:, kc, :],
                                     xc_t[kc][:, t0:t0 + NT],
                                     start=(kc == 0), stop=(kc == 2))
                lsb = gpoolB.tile([3, NT], BF16, tag="lsb", name="lsb")
                nc.scalar.activation(lsb, plg[0:3, :],
                                     mybir.ActivationFunctionType.Identity,
                                     bias=lgb_sb[:, 0:1])
                return lsb

            def gating_part1b(lsb, pool):
                """token-major top-2 softmax math, first half."""
                pltf = pool.tile([HD, NT], BF16, tag="ps", name="plt")
                # 4-col stride keeps each bf16 PSUM write 4-byte aligned
                for t4 in range(4):
                    nc.tensor.transpose(pltf[:, t4 * 4:t4 * 4 + 3],
                                        lsb[:, t4 * HD:(t4 + 1) * HD],
                                        identb[:3, :3])
                lt = gpoolB.tile([HD, 12], F32, tag="lt", name="lt")
                nc.vector.tensor_copy(
                    lt.rearrange("p (g c) -> p g c", c=3),
                    pltf[:, :16].rearrange("p (g c) -> p g c", c=4)[:, :, 0:3])
                l3 = lt.rearrange("p (j e) -> p j e", e=3)
                mx = gpoolB.tile([HD, 4], F32, tag="mx", name="mx")
                nc.vector.tensor_reduce(mx, l3, axis=mybir.AxisListType.X,
                                        op=mybir.AluOpType.max)
                mn = gpoolB.tile([HD, 4], F32, tag="mn", name="mn")
                nc.vector.tensor_reduce(mn, l3, axis=mybir.AxisListType.X,
                                        op=mybir.AluOpType.min)
                sm = gpoolB.tile([HD, 4], F32, tag="sm", name="sm")
                nc.vector.tensor_reduce(sm, l3, axis=mybir.AxisListType.X,
                                        op=mybir.AluOpType.add)
                t1 = gpoolB.tile([HD, 4], F32, tag="t1", name="t1")
                nc.vector.tensor_sub(t1, sm, mx)
                mid = gpoolB.tile([HD, 4], F32, tag="mid", name="mid")
                nc.vector.tensor_sub(mid, t1, mn)
                dm = gpoolB.tile([HD, 4], F32, tag="dm", name="dm")
                nc.vector.tensor_sub(dm, mx, mid)
                th = gpoolB.tile([HD, 4], F32, tag="th", name="th")
                nc.scalar.activation(th, dm,
                                     mybir.ActivationFunctionType.Tanh,
                                     scale=0.5)
                return (lt, mx, mn, th)

            def gating_part1c(st):
                """token-major top-2 softmax math, second half."""
                lt, mx, mn, th = st
                gmx = gpoolB.tile([HD, 4], F32, tag="gmx", name="gmx")
                nc.vector.tensor_scalar(gmx, th, 0.5, 0.5,
                                        op0=mybir.AluOpType.mult,
                                        op1=mybir.AluOpType.add)
                eqx = gpoolB.tile([HD, 12], F32, tag="eqx", name="eqx")
                eqn = gpoolB.tile([HD, 12], F32, tag="eqn", name="eqn")
                for t4 in range(4):
                    sl = slice(t4 * 3, (t4 + 1) * 3)
                    nc.vector.tensor_scalar(eqx[:, sl], lt[:, sl],
                                            mx[:, t4:t4 + 1], None,
                                            op0=mybir.AluOpType.is_equal)
                    nc.vector.tensor_scalar(eqn[:, sl], lt[:, sl],
                                            mn[:, t4:t4 + 1], None,
                                            op0=mybir.AluOpType.is_equal)
                # u = 1 - eqx - eqn (mid indicator); g = gmx*(eqx-u) + u
                s1 = gpoolB.tile([HD, 12], F32, tag="s1", name="s1")
                nc.vector.tensor_add(s1, eqx, eqn)
                u = gpoolB.tile([HD, 12], F32, tag="u", name="u")
                nc.vector.tensor_scalar(u, s1, -1.0, 1.0,
                                        op0=mybir.AluOpType.mult,
                                        op1=mybir.AluOpType.add)
                d0 = gpoolB.tile([HD, 12], F32, tag="d0", name="d0")
                nc.vector.tensor_sub(d0, eqx, u)
                p0 = gpoolB.tile([HD, 12], F32, tag="p0", name="p0")
                for t4 in range(4):
                    sl = slice(t4 * 3, (t4 + 1) * 3)
                    nc.vector.tensor_scalar_mul(p0[:, sl], d0[:, sl],
                                                gmx[:, t4:t4 + 1])
                gm2 = gpoolB.tile([HD, 12], BF16, tag="gm", name="gm")
                nc.vector.tensor_add(gm2, p0, u)
                return gm2

            def gating_part2(gm2, pool):
                """expert-major gates [3, NT] from token-major gm."""
                pgtf = pool.tile([HD, NT], BF16, tag="ps", name="pgt")
                pgt = pgtf[0:3, :]
                for t4 in range(4):
                    nc.tensor.transpose(pgt[:, t4 * HD:(t4 + 1) * HD],
                                        gm2[:, t4 * 3:(t4 + 1) * 3],
                                        identb)
                gates_r = gpoolB.tile([3, NT], BF16, tag="gates",
                                      name="gates_r", bufs=3)
                nc.scalar.copy(gates_r, pgt)
                return gates_r

            def emit_pgb_e(gates_r, e, pspool, tag, t):
                """gate row e broadcast to 128 partitions via one-hot MM."""
                pgb = pspool.tile([HD, NT], F32, tag=tag, name="pgbp")
                nc.tensor.matmul(pgb, wB["eb3"][:, e * HD:(e + 1) * HD],
                                 gates_r, start=True, stop=True)
                pb = gbpool.tile([HD, NT], BF16, tag=f"pgb{e}",
                                 name=f"pgb{e}_{t}")
                nc.scalar.copy(pb, pgb)
                return pb

            g0state = [None]

            # ------------ Phase A: conv MoE + attention per branch --------
            deferred = []
            pend_s2 = []
            s2done = [0]
            gstage = [0]
            with tc.tile_pool(name="gpool", bufs=2) as gpool, \
                 tc.tile_pool(name="psC", bufs=4, space="PSUM") as psC, \
                 tc.tile_pool(name="psT", bufs=4, space="PSUM") as psT:
                for i in range(3):
                    gm = GEOM[i]
                    grs = _groups(i)
                    G = len(grs)
                    if i == 0:
                        xp_sb = br0["xp"]
                        wgd_sb, wca_sb, wcb_sb = \
                            br0["wgd"], br0["wca"], br0["wcb"]
                        wu_sb, wv_sb = br0["wu"], br0["wv"]
                    else:
                        xp_sb = xpool.tile([HD, 6144], BF16, tag="xp")
                        nc.gpsimd.dma_start(out=xp_sb[:, :c1],
                                            in_=xps[i][:, :c1])
                        wgd_sb = wpoolA.tile([HD, HD], BF16, tag="wgd")
                        nc.gpsimd.dma_start(out=wgd_sb, in_=wgd[i])
                        wca_sb = wpoolA.tile([HD, 9, HD], BF16, tag="wca")
                        nc.gpsimd.dma_start(out=wca_sb, in_=wca[i])
                        wcb_sb = wpoolA.tile([HD, 9, HD], BF16, tag="wcb")
                        nc.gpsimd.dma_start(out=wcb_sb, in_=wcb[i])
                        wu_sb = wpoolA.tile([HD, HD], BF16, tag="wu")
                        nc.gpsimd.dma_start(out=wu_sb, in_=wu[i])
                        wv_sb = wpoolA.tile([HD, HD], BF16, tag="wv")
                        nc.gpsimd.dma_start(out=wv_sb, in_=wv[i])
                        nc.gpsimd.dma_start(out=xp_sb[:, c1:c2],
                                            in_=xps[i][:, c1:c2])
                        nc.sync.dma_start(out=xp_sb[:, c2:gm["BUF"]],
                                          in_=xps[i][:, c2:])
                    if i == 1:
                        emit_phaseB_weight_dmas()

                    # max used extent: branch 1 (lo=768, rlen=4608 -> 5376)
                    moe_buf = mpool.tile([HD, 5376], BF16, tag="moe")
                    u_buf = mpool.tile([HD, 5376], BF16, tag="u")

                    st = {}

                    def conv_a(g):
                        fo, n = grs[g]
                        plg = psC.tile([HD, NT], F32, tag="ps", name="plg")
                        nc.tensor.matmul(plg[:, :n], wgd_sb,
                                         xp_sb[:, fo:fo + n],
                                         start=True, stop=True)
                        ex = gpool.tile([HD, NT], BF16, tag="ex")
                        nc.scalar.activation(ex[:, :n], plg[:, :n],
                                             mybir.ActivationFunctionType.Tanh,
                                             scale=-0.5)
                        pa = psC.tile([HD, NT], F32, tag="ps", name="pa")
                        for ti, (dr, ds) in enumerate(TAPS_A[i]):
                            o = dr * gm["SP"] + ds
                            nc.tensor.matmul(pa[:, :n], wca_sb[:, ti, :],
                                             xp_sb[:, fo + o: fo + o + n],
                                             start=(ti == 0), stop=(ti == 8))
                        st[g] = (ex, pa)

                    def conv_b(g):
                        fo, n = grs[g]
                        pb = psC.tile([HD, NT], F32, tag="ps", name="pb")
                        for ti, (dr, ds) in enumerate(TAPS_B[i]):
                            o = dr * gm["SP"] + ds
                            nc.tensor.matmul(pb[:, :n], wcb_sb[:, ti, :],
                                             xp_sb[:, fo + o: fo + o + n],
                                             start=(ti == 0), stop=(ti == 8))
                        st[g] = st[g] + (pb,)

                    def moe_math(g):
                        fo, n = grs[g]
                        ex, pa, pb = st.pop(g)
                        ca = gpool.tile([HD, NT], BF16, tag="ca")
                        nc.scalar.activation(ca[:, :n], pa[:, :n],
                                             mybir.ActivationFunctionType.Identity,
                                             bias=bca_sb[:, i:i + 1], scale=0.5)
                        cb = gpool.tile([HD, NT], BF16, tag="cb")
                        nc.scalar.activation(cb[:, :n], pb[:, :n],
                                             mybir.ActivationFunctionType.Identity,
                                             bias=bcb_sb[:, i:i + 1], scale=0.5)
                        dd = gpool.tile([HD, NT], BF16, tag="dd")
                        nc.vector.tensor_sub(dd[:, :n], ca[:, :n], cb[:, :n])
                        d2 = gpool.tile([HD, NT], BF16, tag="d2")
                        nc.vector.tensor_mul(d2[:, :n], dd[:, :n], ex[:, :n])
                        ss = gpool.tile([HD, NT], BF16, tag="ss")
                        nc.vector.tensor_add(ss[:, :n], ca[:, :n], cb[:, :n])
                        nc.vector.tensor_add(moe_buf[:, fo:fo + n],
                                             ss[:, :n], d2[:, :n])

                    def qk(g):
                        fo, n = grs[g]
                        pq = psT.tile([HD, NT], F32, tag="ps", name="pq")
                        nc.tensor.matmul(pq[:, :n], wu_sb,
                                         moe_buf[:, fo:fo + n],
                                         start=True, stop=True)
                        nc.scalar.copy(u_buf[:, fo:fo + n], pq[:, :n])

                    def attn_s1(a, pspool=psT, pscpool=None, psctag="ps"):
                        if pscpool is None:
                            pscpool = pspool
                        offs = [_row_off(i, 4 * a + j) for j in range(4)]
                        pvt = pspool.tile([96, 4 * HD], F32, tag="ps", name="pvt")
                        for j in range(4):
                            nc.tensor.matmul(pvt[:, j * HD:(j + 1) * HD],
                                             moe_buf[:, offs[j]:offs[j] + 96],
                                             wv_sb, start=True, stop=True)
                        vt = apool.tile([96, 4 * HD], BF16, tag="vt")
                        nc.vector.tensor_copy(vt, pvt)
                        psc = pscpool.tile([96, GN], F32, tag=psctag,
                                           name="psc")
                        for j in range(4):
                            nc.tensor.matmul(psc[:, j * 96:(j + 1) * 96],
                                             u_buf[:, offs[j]:offs[j] + 96],
                                             moe_buf[:, offs[j]:offs[j] + 96],
                                             start=True, stop=True)
                        probs = apool.tile([96, GN], BF16, tag="probs")
                        nc.scalar.activation(probs, psc,
                                             mybir.ActivationFunctionType.Exp,
                                             scale=SCALE)
                        zsum = apool.tile([96, 4], F32, tag="zsum")
                        nc.vector.tensor_reduce(
                            zsum, probs.rearrange("p (j q) -> p j q", q=96),
                            axis=mybir.AxisListType.X, op=mybir.AluOpType.add)
                        rec = apool.tile([96, 4], F32, tag="rec")
                        nc.vector.reciprocal(rec, zsum)
                        pn = apool.tile([96, GN], BF16, tag="pn")
                        for j in range(4):
                            nc.vector.tensor_scalar_mul(
                                pn[:, j * 96:(j + 1) * 96],
                                probs[:, j * 96:(j + 1) * 96],
                                rec[:, j:j + 1])
                        return (a, vt, pn)

                    def attn_s2(s, pspool=psT, ptag="ps", i=i):
                        a, vt, pn = s
                        ppt = pspool.tile([96, GN], BF16, tag=ptag, name="ppt")
                        for j in range(4):
                            nc.tensor.transpose(ppt[:, j * 96:(j + 1) * 96],
                                                pn[:, j * 96:(j + 1) * 96],
                                                identb[:96, :96])
                        pt = apool.tile([96, GN], BF16, tag="pt")
                        nc.vector.tensor_copy(pt, ppt)
                        po = pspool.tile([HD, GN], F32, tag=ptag, name="po")
                        for j in range(4):
                            nc.tensor.matmul(po[:, j * 96:(j + 1) * 96],
                                             vt[:, j * HD:(j + 1) * HD],
                                             pt[:, j * 96:(j + 1) * 96],
                                             start=True, stop=True)
                        nc.scalar.copy(xc_t[i][:, a * GN:(a + 1) * GN], po)

                    conv_a(0)
                    conv_b(0)
                    a_next = 0
                    done = 0
                    for g in range(G):
                        if g + 1 < G:
                            conv_a(g + 1)
                        # drain at most one pending s2 here; the rest after
                        # conv_b so the softmax vector chain has more slack
                        if len(pend_s2) > 2:
                            bi, f2, s2 = pend_s2.pop(0)
                            f2(s2)
                            if bi == 2:
                                s2done[0] += 1
                        # tiles 0/1's gating hoisted under branch 2's conv
                        # stream as soon as the needed xc columns exist, so
                        # phase B starts with gates two tiles ahead
                        if i == 2 and s2done[0] >= 3 and gstage[0] == 0:
                            gstage[0] = 1
                            tail_lsb0 = gating_part1a(0, psT)
                            tail_gm0 = gating_part1c(
                                gating_part1b(tail_lsb0, psT))
                        elif i == 2 and s2done[0] >= 4 and gstage[0] == 1:
                            gstage[0] = 2
                            tail_g0 = gating_part2(tail_gm0, psT)
                            tail_pgbs0 = [emit_pgb_e(tail_g0, e, psT, "ps", 0)
                                          for e in range(3)]
                            tail_lsb1 = gating_part1a(1, psT)
                        elif i == 2 and s2done[0] >= 5 and gstage[0] == 2:
                            gstage[0] = 3
                            tail_g1 = gating_part2(gating_part1c(
                                gating_part1b(tail_lsb1, psT)), psT)
                        moe_math(g)
                        if g + 1 < G:
                            conv_b(g + 1)
                        while len(pend_s2) > 2:
                            bi, f2, s2 = pend_s2.pop(0)
                            f2(s2)
                            if bi == 2:
                                s2done[0] += 1
                        qk(g)
                        done += grs[g][1]
                        cov = gm["lo"] + done
                        amax = 6 if i == 2 else 12
                        while a_next < amax and \
                                _row_off(i, 4 * a_next + 3) + 96 <= cov:
                            pend_s2.append((i, attn_s2, attn_s1(a_next)))
                            a_next += 1
                    # branch tail s2's carry into the next branch's conv
                    # stream; branch 2 drains fully before the phase-B tail.
                    if i == 2:
                        while pend_s2:
                            bi, f2, s2 = pend_s2.pop(0)
                            f2(s2)
                            if bi == 2:
                                s2done[0] += 1
                        for a in range(6, 12):
                            deferred.append((attn_s1, attn_s2, a))
                # fallback if the hoist conditions never fired late enough
                if gstage[0] < 3:
                    if gstage[0] == 0:
                        tail_lsb0 = gating_part1a(0, psT)
                        tail_gm0 = gating_part1c(
                            gating_part1b(tail_lsb0, psT))
                        gstage[0] = 1
                    if gstage[0] == 1:
                        tail_g0 = gating_part2(tail_gm0, psT)
                        tail_pgbs0 = [emit_pgb_e(tail_g0, e, psT, "ps", 0)
                                      for e in range(3)]
                        tail_lsb1 = gating_part1a(1, psT)
                        gstage[0] = 2
                    tail_g1 = gating_part2(gating_part1c(
                        gating_part1b(tail_lsb1, psT)), psT)
                    gstage[0] = 3
                g0state[0] = (tail_g0, tail_pgbs0, tail_g1)
            stackX.close()

            # ---------------- Phase B: final MLP MoE + proj ---------------
            with tc.tile_pool(name="bpool", bufs=3) as bpool, \
                 tc.tile_pool(name="hpool", bufs=5) as hpool, \
                 tc.tile_pool(name="psL", bufs=3, space="PSUM") as psL, \
                 tc.tile_pool(name="psGB", bufs=1, space="PSUM") as psGB, \
                 tc.tile_pool(name="psPG", bufs=1, space="PSUM") as psPG, \
                 tc.tile_pool(name="psB", bufs=3, space="PSUM") as psB:
                w1_sb = wB["w1"]
                w2_sb = wB["w2"]
                b1_sb = wB["b1"]
                b2r_sb = wB["b2r"]
                bpr_sb = wB["bpr"]

                LA = 4
                iters = [(e, m) for e in range(3) for m in range(12)]
                gates_cur, pgbs_cur, gates_next = g0state[0]
                lsb_n2 = None
                gates_n2 = None
                dpend = []
                # deferred branch-2 attention spread over tiles 0-2
                DEF_SCHED = {(0, 4): (1, 0), (0, 14): (2, 0),
                             (0, 20): (1, 1), (0, 29): (2, 1),
                             (1, 4): (1, 2), (1, 14): (2, 2),
                             (1, 20): (1, 3), (1, 29): (2, 3),
                             (2, 4): (1, 4), (2, 14): (2, 4),
                             (2, 20): (1, 5), (2, 29): (2, 5)}
                drain_prev = [None]

                def emit_drain(final=False):
                    if drain_prev[0] is None:
                        return
                    pdp, tp0 = drain_prev[0]
                    drain_prev[0] = None
                    if final:
                        # fan the last drain out across engines/queues so the
                        # tail isn't serialized behind one scalar+DMA chain
                        osb0 = bpool.tile([HD, NT], F32, tag="osb")
                        nc.scalar.activation(
                            osb0, pdp[0], mybir.ActivationFunctionType.Identity,
                            bias=bpr_sb[:, 0:1])
                        nc.sync.dma_start(out=out_cm[0:HD, tp0:tp0 + NT],
                                          in_=osb0)
                        osb1 = bpool.tile([HD, NT], F32, tag="osb")
                        nc.vector.tensor_scalar_add(osb1, pdp[1],
                                                    bpr_sb[:, 1:2])
                        nc.scalar.dma_start(
                            out=out_cm[HD:2 * HD, tp0:tp0 + NT], in_=osb1)
                        osb2 = bpool.tile([HD, NT], F32, tag="osb")
                        nc.vector.tensor_scalar_add(osb2, pdp[2],
                                                    bpr_sb[:, 2:3])
                        nc.gpsimd.dma_start(
                            out=out_cm[2 * HD:3 * HD, tp0:tp0 + NT], in_=osb2)
                        return
                    for mp in range(3):
                        osb = bpool.tile([HD, NT], F32, tag="osb")
                        nc.scalar.activation(
                            osb, pdp[mp],
                            mybir.ActivationFunctionType.Identity,
                            bias=bpr_sb[:, mp:mp + 1])
                        nc.sync.dma_start(
                            out=out_cm[mp * HD:(mp + 1) * HD, tp0:tp0 + NT],
                            in_=osb)

                for t in range(NTILES):
                    t0 = t * NT
                    pd = [psL.tile([HD, NT], F32, tag="down", name=f"pd{_i}")
                          for _i in range(3)]
                    hs_l = {}
                    for k in range(36 + LA):
                        if k < 36:
                            e, m = iters[k]
                            pu = psB.tile([HD, NT], F32, tag="ps", name="pu")
                            for kc in range(3):
                                nc.tensor.matmul(
                                    pu, w1_sb[e][:, kc, m * HD:(m + 1) * HD],
                                    xc_t[kc][:, t0:t0 + NT],
                                    start=(kc == 0), stop=(kc == 2))
                            h = hpool.tile([HD, NT], BF16, tag="h")
                            nc.scalar.activation(
                                h, pu, mybir.ActivationFunctionType.Gelu,
                                bias=b1_sb[:, e, m:m + 1])
                            hs = hpool.tile([HD, NT], BF16, tag="hs")
                            nc.vector.tensor_mul(hs, h, pgbs_cur[e])
                            hs_l[k] = (e, m, hs)
                            if k == 1:
                                emit_drain()
                            if (t, k) in DEF_SCHED:
                                which, idx = DEF_SCHED[(t, k)]
                                s1f, s2f, a = deferred[idx]
                                if which == 1:
                                    dpend.append(s1f(a, psGB, psPG, "pgb"))
                                else:
                                    s2f(dpend.pop(0), psPG, "pgb")
                            if (e, m) == (0, 0) and t + 2 < NTILES:
                                lsb_n2 = gating_part1a(t + 2, psGB)
                            if (e, m) == (0, 6) and t + 2 < NTILES:
                                st_n2 = gating_part1b(lsb_n2, psGB)
                            if (e, m) == (0, 9) and t + 2 < NTILES:
                                gm_n2 = gating_part1c(st_n2)
                            if (e, m) == (1, 0) and t + 2 < NTILES:
                                gates_n2 = gating_part2(gm_n2, psGB)
                            if e == 2 and m in (7, 9, 11) and t + 1 < NTILES:
                                e_ = (m - 7) // 2
                                pb = emit_pgb_e(gates_next, e_, psPG, "pgb",
                                                t + 1)
                                if e_ == 0:
                                    pgbs_next = []
                                pgbs_next.append(pb)
                        if k >= LA:
                            e2, m2, hs2 = hs_l.pop(k - LA)
                            for mp in range(3):
                                nc.tensor.matmul(
                                    pd[mp],
                                    w2_sb[e2][:, m2, mp * HD:(mp + 1) * HD],
                                    hs2, start=(e2 == 0 and m2 == 0),
                                    stop=False)
                    for mp in range(3):
                        nc.tensor.matmul(pd[mp],
                                         b2r_sb[:, mp * HD:(mp + 1) * HD],
                                         gates_cur, start=False, stop=True)
                    drain_prev[0] = (pd, t0)
                    if t + 1 < NTILES:
                        gates_cur, pgbs_cur = gates_next, pgbs_next
                        gates_next = gates_n2
                emit_drain(final=True)
            stackA.close()
    nc.compile()
    return nc


def _prep_inputs(x, w_e1, b_e1, w_e2, b_e2, w_e3, b_e3, w_e4, b_e4, w_e5, b_e5,
                 w_e6, b_e6, wg1, wg2, wg3, w_qkv, w_attn_proj, b_attn_proj,
                 wg_final, w_mlp1, b_mlp1, w_mlp2, b_mlp2, w_proj, b_proj):
    f = np.float32
    shared = {}
    shared["wca"] = np.ascontiguousarray(np.stack([
        w_e1.reshape(9, HD, HD), w_e3.reshape(9, HD, HD),
        w_e5.reshape(9, HD, HD)]).transpose(0, 2, 1, 3), dtype=f).astype(BFNP)
    shared["wcb"] = np.ascontiguousarray(np.stack([
        w_e2.reshape(9, HD, HD), w_e4.reshape(9, HD, HD),
        w_e6.reshape(9, HD, HD)]).transpose(0, 2, 1, 3), dtype=f).astype(BFNP)
    shared["bca"] = np.ascontiguousarray(
        np.stack([b_e1, b_e3, b_e5], axis=1) * 0.5, dtype=f)
    shared["bcb"] = np.ascontiguousarray(
        np.stack([b_e2, b_e4, b_e6], axis=1) * 0.5, dtype=f)
    wgs = np.stack([wg1, wg2, wg3])
    shared["wgd"] = np.ascontiguousarray(
        np.repeat((wgs[:, :, 1] - wgs[:, :, 0])[:, :, None], HD, axis=2),
        dtype=f).astype(BFNP)
    eb3 = np.zeros((3, 384), f)
    for e in range(3):
        eb3[e, e * 128:(e + 1) * 128] = 1.0
    shared["eb3"] = eb3.astype(BFNP)
    wq64 = np.asarray(w_qkv[:, :, :HD], dtype=np.float64)
    wk64 = np.asarray(w_qkv[:, :, HD:256], dtype=np.float64)
    shared["wu"] = np.ascontiguousarray(
        np.einsum("icq,idq->icd", wq64, wk64), dtype=f).astype(BFNP)
    wv64 = np.asarray(w_qkv[:, :, 256:], dtype=np.float64)
    wap64 = np.asarray(w_attn_proj, dtype=np.float64)
    shared["wv"] = np.ascontiguousarray(
        np.einsum("ick,iko->ico", wv64, wap64), dtype=f).astype(BFNP)
    # fold the attention-proj bias into the MLP/gate paths (xc on device
    # is stored without it): b1' = b1 + bap @ w1; lgb = bap @ wg_final
    bap64 = np.asarray(b_attn_proj, np.float64).reshape(C)
    shared["lgb"] = np.tile(
        (bap64 @ np.asarray(wg_final, np.float64)).reshape(3, 1),
        (1, 4)).astype(f)
    shared["wgf"] = np.ascontiguousarray(
        np.tile(wg_final.reshape(3, HD, 3), (1, 1, 43))[:, :, :HD],
        dtype=f).astype(BFNP)
    shared["w1"] = np.ascontiguousarray(
        w_mlp1.reshape(3, 3, HD, 1536).transpose(0, 2, 1, 3),
        dtype=f).astype(BFNP)
    b1p = np.asarray(b_mlp1, np.float64) + \
        np.einsum("c,ecf->ef", bap64, np.asarray(w_mlp1, np.float64))
    shared["b1"] = np.ascontiguousarray(
        b1p.reshape(3, 12, HD).transpose(2, 0, 1), dtype=f)
    w2p = np.asarray(w_mlp2, dtype=np.float64) @ np.asarray(w_proj, np.float64)
    shared["w2"] = np.ascontiguousarray(
        w2p.reshape(3, 12, HD, C).transpose(0, 2, 1, 3), dtype=f).astype(BFNP)
    shared["b2r"] = np.ascontiguousarray(
        np.asarray(b_mlp2, np.float64) @ np.asarray(w_proj, np.float64),
        dtype=f).astype(BFNP)
    shared["bpr"] = np.ascontiguousarray(b_proj.reshape(3, HD).T, dtype=f)

    in_maps = []
    xf = np.asarray(x, dtype=f)
    for c in range(N_CORES):
        b, halfc = c // 2, c % 2
        r0 = halfc * R
        m = dict(shared)
        for i in range(3):
            g = GEOM[i]
            xi = xf[b, :, :, i * HD:(i + 1) * HD]  # [96, 96, 128]
            plane = np.zeros((HD, g["NR"], g["SP"]), f)
            glo = max(0, r0 - g["pad_r"])
            ghi = min(HH, r0 + R + g["pad_r"])
            plo = glo - (r0 - g["pad_r"])
            plane[:, plo:plo + (ghi - glo),
                  g["pad_c"]:g["pad_c"] + 96] = \
                xi[glo:ghi].transpose(2, 0, 1)
            buf = np.zeros((HD, g["BUF"]), f)
            buf[:, g["OFF"]:g["OFF"] + g["NR"] * g["SP"]] = \
                plane.reshape(HD, -1)
            m[f"xp{i}"] = buf.astype(BFNP)
        m["xp2"][:, GEOM[2]["OFF"] + GEOM[2]["NR"] * GEOM[2]["SP"]:] = 0
        in_maps.append(m)
    return in_maps


def kernel(**inputs):
    global _CACHED_NC
    if _CACHED_NC is None:
        _CACHED_NC = build_kernel()
    nc = _CACHED_NC
    in_maps = _prep_inputs(**{k: np.asarray(v) for k, v in inputs.items()})
    res = None
    for attempt in range(3):
        try:
            res = run_bass_kernel_spmd(nc, in_maps,
                                       core_ids=list(range(N_CORES)))
            break
        except Exception:
            if attempt == 2:
                raise
            import time
            time.sleep(2.0)
    out = np.empty((B, HH, WW, C), np.float32)
    for c in range(N_CORES):
        b, halfc = c // 2, c % 2
        slab = res.results[c]["out_cm"].reshape(C, R, 96)
        out[b, :, halfc * R:(halfc + 1) * R, :] = slab.transpose(2, 1, 0)
    return out



# revision 70
# speedup vs baseline: 1.0034x; 1.0034x over previous
"""Trainium2 Bass kernel for nn_MAMoE (conv-MoE -> row attention -> MLP-MoE).

Sharding: 8 cores = (batch b in 0..3) x (H-half in 0..1). All routing is
per-token; the reference's swapaxes(1,2) means attention row r produces
output column w=r, so each core independently computes the full pipeline
for its 48 attention rows and the host reassembles along W.

v3: all matmuls bf16 (f32 PSUM accumulate); per-branch minimal conv
padding (100/96/112 row pitch); two-stage attention emission so the
softmax chain hides under the next conv group's matmuls; software-
pipelined phase B (ups run 2 iterations ahead of downs); first tile's
gating hoisted into the phase A tail.
"""
import contextlib

import numpy as np
import ml_dtypes

import concourse.bass as bass
import concourse.bass_isa as bass_isa
import concourse.mybir as mybir
import concourse.tile as tile
from concourse import bacc
from concourse.bass_utils import run_bass_kernel_spmd
from concourse.masks import make_identity

F32 = mybir.dt.float32
BF16 = mybir.dt.bfloat16
BFNP = ml_dtypes.bfloat16

B, HH, WW, C = 4, 96, 96, 384
HD = 128
SCALE = float((HD // 3) ** -0.5)  # 42**-0.5
N_CORES = 8
R = 48            # attention rows per core
T = R * 96        # tokens per core = 4608
NT = 512          # tokens per MLP tile
NTILES = T // NT  # 9
GN = 4 * 96       # tokens per attention group = 384

# per-branch padded-plane geometry
GEOM = [
    dict(SP=100, NR=52, OFF=8, BUF=5248, pad_r=2, pad_c=2),   # 3x3 convs
    dict(SP=96, NR=64, OFF=0, BUF=6144, pad_r=8, pad_c=0),    # (9,1) convs
    dict(SP=104, NR=48, OFF=8, BUF=5120, pad_r=0, pad_c=0),   # (1,9) convs
]
for _g in GEOM:
    _g["lo"] = _g["OFF"] + _g["pad_r"] * _g["SP"]
    _g["rlen"] = 48 * _g["SP"]
GEOM[2]["rlen"] = 47 * 104 + 96

TAPS_A = [
    [(dr, ds) for dr in (-1, 0, 1) for ds in (-1, 0, 1)],
    [(dr, 0) for dr in range(-4, 5)],
    [(0, ds) for ds in range(-4, 5)],
]
TAPS_B = [
    [(dr, ds) for dr in (-2, 0, 2) for ds in (-2, 0, 2)],
    [(dr, 0) for dr in range(-8, 9, 2)],
    [(0, ds) for ds in range(-8, 9, 2)],
]


def _row_off(i, r):
    g = GEOM[i]
    return g["OFF"] + (g["pad_r"] + r) * g["SP"] + g["pad_c"]


def _groups(i):
    g = GEOM[i]
    out = []
    fo = g["lo"]
    end = g["lo"] + g["rlen"]
    while fo < end:
        out.append((fo, min(NT, end - fo)))
        fo += NT
    return out


_CACHED_NC = None


def build_kernel():
    nc = bacc.Bacc("TRN2", target_bir_lowering=False, debug=False)

    xps = [nc.dram_tensor(f"xp{i}", [HD, GEOM[i]["BUF"]], BF16,
                          kind="ExternalInput").ap() for i in range(3)]
    wca = nc.dram_tensor("wca", [3, HD, 9, HD], BF16, kind="ExternalInput").ap()
    wcb = nc.dram_tensor("wcb", [3, HD, 9, HD], BF16, kind="ExternalInput").ap()
    bca = nc.dram_tensor("bca", [HD, 3], F32, kind="ExternalInput").ap()
    bcb = nc.dram_tensor("bcb", [HD, 3], F32, kind="ExternalInput").ap()
    wgd = nc.dram_tensor("wgd", [3, HD, HD], BF16, kind="ExternalInput").ap()
    eb3 = nc.dram_tensor("eb3", [3, 384], BF16, kind="ExternalInput").ap()
    wu = nc.dram_tensor("wu", [3, HD, HD], BF16, kind="ExternalInput").ap()
    wv = nc.dram_tensor("wv", [3, HD, HD], BF16, kind="ExternalInput").ap()
    lgb = nc.dram_tensor("lgb", [3, 4], F32, kind="ExternalInput").ap()
    wgf = nc.dram_tensor("wgf", [3, HD, HD], BF16, kind="ExternalInput").ap()
    w1 = nc.dram_tensor("w1", [3, HD, 3, 1536], BF16, kind="ExternalInput").ap()
    b1 = nc.dram_tensor("b1", [HD, 3, 12], F32, kind="ExternalInput").ap()
    w2 = nc.dram_tensor("w2", [3, HD, 12, C], BF16, kind="ExternalInput").ap()
    b2r = nc.dram_tensor("b2r", [3, C], BF16, kind="ExternalInput").ap()
    out_cm = nc.dram_tensor("out_cm", [C, T], F32, kind="ExternalOutput").ap()

    with tile.TileContext(nc) as tc:
        with tc.tile_pool(name="persist", bufs=1) as persist, \
             tc.tile_pool(name="wpoolB", bufs=1) as wpoolB, \
             tc.tile_pool(name="gbpool", bufs=2) as gbpool, \
             tc.tile_pool(name="gpoolB", bufs=2) as gpoolB:
            stackA = contextlib.ExitStack()
            mpool = stackA.enter_context(tc.tile_pool(name="mpool", bufs=2))
            wpoolA = stackA.enter_context(tc.tile_pool(name="wpoolA", bufs=2))
            apool = stackA.enter_context(tc.tile_pool(name="apool", bufs=4))
            anorm = stackA.enter_context(tc.tile_pool(name="anorm", bufs=2))
            stackX = contextlib.ExitStack()
            xpool = stackX.enter_context(tc.tile_pool(name="xpool", bufs=2))
            # ---- branch-0 critical DMAs first (minimize first-matmul wait);
            # spread the first transfers across 4 queues so their fixed DMA
            # latencies overlap.
            xp_sb0 = xpool.tile([HD, 6144], BF16, tag="xp")
            c1, c2 = 1024, 3072
            nc.sync.dma_start(out=xp_sb0[:, :c1], in_=xps[0][:, :c1])
            wgd_sb0 = wpoolA.tile([HD, HD], BF16, tag="wgd")
            nc.scalar.dma_start(out=wgd_sb0, in_=wgd[0])
            wca_sb0 = wpoolA.tile([HD, 9, HD], BF16, tag="wca")
            nc.gpsimd.dma_start(out=wca_sb0, in_=wca[0])
            wcb_sb0 = wpoolA.tile([HD, 9, HD], BF16, tag="wcb")
            nc.gpsimd.dma_start(out=wcb_sb0, in_=wcb[0])
            wu_sb0 = wpoolA.tile([HD, HD], BF16, tag="wu")
            nc.gpsimd.dma_start(out=wu_sb0, in_=wu[0])
            wv_sb0 = wpoolA.tile([HD, HD], BF16, tag="wv")
            nc.gpsimd.dma_start(out=wv_sb0, in_=wv[0])
            nc.gpsimd.dma_start(out=xp_sb0[:, c1:c2], in_=xps[0][:, c1:c2])
            nc.sync.dma_start(out=xp_sb0[:, c2:GEOM[0]["BUF"]],
                              in_=xps[0][:, c2:])
            bca_sb = persist.tile([HD, 3], F32)
            bcb_sb = persist.tile([HD, 3], F32)
            lgb_sb = persist.tile([3, 4], F32)
            nc.sync.dma_start(out=bca_sb, in_=bca)
            nc.sync.dma_start(out=bcb_sb, in_=bcb)
            nc.sync.dma_start(out=lgb_sb, in_=lgb)

            br0 = dict(xp=xp_sb0, wgd=wgd_sb0, wca=wca_sb0, wcb=wcb_sb0,
                       wu=wu_sb0, wv=wv_sb0)

            identb = persist.tile([HD, HD], BF16)
            make_identity(nc, identb)

            xc_t = [persist.tile([HD, T], BF16, tag=f"xc{i}", name=f"xc{i}")
                    for i in range(3)]

            wB = {}

            def emit_phaseB_weight_dmas():
                wB["b1"] = wpoolB.tile([HD, 3, 12], F32, tag="b1", name="b1s")
                nc.sync.dma_start(out=wB["b1"], in_=b1)
                wB["b2r"] = wpoolB.tile([3, C], BF16, tag="b2r", name="b2rs")
                nc.sync.dma_start(out=wB["b2r"], in_=b2r)
                wB["wgf"] = wpoolB.tile([HD, 3, HD], BF16, tag="wgf", name="wgfs")
                nc.sync.dma_start(out=wB["wgf"],
                                  in_=wgf.rearrange("a p b -> p a b"))
                wB["eb3"] = wpoolB.tile([3, 384], BF16, tag="eb3", name="eb3s")
                nc.sync.dma_start(out=wB["eb3"], in_=eb3)
                wB["w1"] = []
                wB["w2"] = []
                for e in range(3):
                    t1 = wpoolB.tile([HD, 3, 1536], BF16, tag=f"w1_{e}",
                                     name=f"w1_{e}")
                    nc.sync.dma_start(out=t1, in_=w1[e])
                    wB["w1"].append(t1)
                    t2 = wpoolB.tile([HD, 12, C], BF16, tag=f"w2_{e}",
                                     name=f"w2_{e}")
                    nc.sync.dma_start(out=t2, in_=w2[e])
                    wB["w2"].append(t2)

            def gating_part1a(t, pool):
                """logits matmul for tile t."""
                t0 = t * NT
                plg = pool.tile([HD, NT], F32, tag="ps", name="plg")
                for kc in range(3):
                    nc.tensor.matmul(plg, wB["wgf"][:, kc, :],
                                     xc_t[kc][:, t0:t0 + NT],
                                     start=(kc == 0), stop=(kc == 2))
                lsb = gpoolB.tile([3, NT], BF16, tag="lsb", name="lsb")
                nc.scalar.activation(lsb, plg[0:3, :],
                                     mybir.ActivationFunctionType.Identity,
                                     bias=lgb_sb[:, 0:1])
                return lsb

            def gating_part1b(lsb, pool):
                """token-major top-2 softmax math, first half."""
                pltf = pool.tile([HD, NT], BF16, tag="ps", name="plt")
                # 4-col stride keeps each bf16 PSUM write 4-byte aligned
                for t4 in range(4):
                    nc.tensor.transpose(pltf[:, t4 * 4:t4 * 4 + 3],
                                        lsb[:, t4 * HD:(t4 + 1) * HD],
                                        identb[:3, :3])
                lt = gpoolB.tile([HD, 12], F32, tag="lt", name="lt")
                nc.vector.tensor_copy(
                    lt.rearrange("p (g c) -> p g c", c=3),
                    pltf[:, :16].rearrange("p (g c) -> p g c", c=4)[:, :, 0:3])
                l3 = lt.rearrange("p (j e) -> p j e", e=3)
                mx = gpoolB.tile([HD, 4], F32, tag="mx", name="mx")
                nc.vector.tensor_reduce(mx, l3, axis=mybir.AxisListType.X,
                                        op=mybir.AluOpType.max)
                mn = gpoolB.tile([HD, 4], F32, tag="mn", name="mn")
                nc.vector.tensor_reduce(mn, l3, axis=mybir.AxisListType.X,
                                        op=mybir.AluOpType.min)
                sm = gpoolB.tile([HD, 4], F32, tag="sm", name="sm")
                nc.vector.tensor_reduce(sm, l3, axis=mybir.AxisListType.X,
                                        op=mybir.AluOpType.add)
                t1 = gpoolB.tile([HD, 4], F32, tag="t1", name="t1")
                nc.vector.tensor_sub(t1, sm, mx)
                mid = gpoolB.tile([HD, 4], F32, tag="mid", name="mid")
                nc.vector.tensor_sub(mid, t1, mn)
                dm = gpoolB.tile([HD, 4], F32, tag="dm", name="dm")
                nc.vector.tensor_sub(dm, mx, mid)
                th = gpoolB.tile([HD, 4], F32, tag="th", name="th")
                nc.scalar.activation(th, dm,
                                     mybir.ActivationFunctionType.Tanh,
                                     scale=0.5)
                return (lt, mx, mn, th)

            def gating_part1c(st):
                """token-major top-2 softmax math, second half."""
                lt, mx, mn, th = st
                gmx = gpoolB.tile([HD, 4], F32, tag="gmx", name="gmx")
                nc.vector.tensor_scalar(gmx, th, 0.5, 0.5,
                                        op0=mybir.AluOpType.mult,
                                        op1=mybir.AluOpType.add)
                eqx = gpoolB.tile([HD, 12], F32, tag="eqx", name="eqx")
                eqn = gpoolB.tile([HD, 12], F32, tag="eqn", name="eqn")
                for t4 in range(4):
                    sl = slice(t4 * 3, (t4 + 1) * 3)
                    nc.vector.tensor_scalar(eqx[:, sl], lt[:, sl],
                                            mx[:, t4:t4 + 1], None,
                                            op0=mybir.AluOpType.is_equal)
                    nc.vector.tensor_scalar(eqn[:, sl], lt[:, sl],
                                            mn[:, t4:t4 + 1], None,
                                            op0=mybir.AluOpType.is_equal)
                # u = 1 - eqx - eqn (mid indicator); g = gmx*(eqx-u) + u
                s1 = gpoolB.tile([HD, 12], F32, tag="s1", name="s1")
                nc.vector.tensor_add(s1, eqx, eqn)
                u = gpoolB.tile([HD, 12], F32, tag="u", name="u")
                nc.vector.tensor_scalar(u, s1, -1.0, 1.0,
                                        op0=mybir.AluOpType.mult,
                                        op1=mybir.AluOpType.add)
                d0 = gpoolB.tile([HD, 12], F32, tag="d0", name="d0")
                nc.vector.tensor_sub(d0, eqx, u)
                p0 = gpoolB.tile([HD, 12], F32, tag="p0", name="p0")
                for t4 in range(4):
                    sl = slice(t4 * 3, (t4 + 1) * 3)
                    nc.vector.tensor_scalar_mul(p0[:, sl], d0[:, sl],
                                                gmx[:, t4:t4 + 1])
                gm2 = gpoolB.tile([HD, 12], BF16, tag="gm", name="gm")
                nc.vector.tensor_add(gm2, p0, u)
                return gm2

            def gating_part2(gm2, pool):
                """expert-major gates [3, NT] from token-major gm."""
                pgtf = pool.tile([HD, NT], BF16, tag="ps", name="pgt")
                pgt = pgtf[0:3, :]
                for t4 in range(4):
                    nc.tensor.transpose(pgt[:, t4 * HD:(t4 + 1) * HD],
                                        gm2[:, t4 * 3:(t4 + 1) * 3],
                                        identb)
                gates_r = gpoolB.tile([3, NT], BF16, tag="gates",
                                      name="gates_r", bufs=3)
                nc.scalar.copy(gates_r, pgt)
                return gates_r

            def emit_pgb_e(gates_r, e, pspool, tag, t):
                """gate row e broadcast to 128 partitions via one-hot MM."""
                pgb = pspool.tile([HD, NT], F32, tag=tag, name="pgbp")
                nc.tensor.matmul(pgb, wB["eb3"][:, e * HD:(e + 1) * HD],
                                 gates_r, start=True, stop=True)
                pb = gbpool.tile([HD, NT], BF16, tag=f"pgb{e}",
                                 name=f"pgb{e}_{t}")
                nc.scalar.copy(pb, pgb)
                return pb

            g0state = [None]

            # ------------ Phase A: conv MoE + attention per branch --------
            deferred = []
            pend_s2 = []
            s2done = [0]
            gstage = [0]
            with tc.tile_pool(name="gpool", bufs=2) as gpool, \
                 tc.tile_pool(name="psC", bufs=4, space="PSUM") as psC, \
                 tc.tile_pool(name="psT", bufs=4, space="PSUM") as psT:
                for i in range(3):
                    gm = GEOM[i]
                    grs = _groups(i)
                    G = len(grs)
                    if i == 0:
                        xp_sb = br0["xp"]
                        wgd_sb, wca_sb, wcb_sb = \
                            br0["wgd"], br0["wca"], br0["wcb"]
                        wu_sb, wv_sb = br0["wu"], br0["wv"]
                    else:
                        xp_sb = xpool.tile([HD, 6144], BF16, tag="xp")
                        nc.gpsimd.dma_start(out=xp_sb[:, :c1],
                                            in_=xps[i][:, :c1])
                        wgd_sb = wpoolA.tile([HD, HD], BF16, tag="wgd")
                        nc.gpsimd.dma_start(out=wgd_sb, in_=wgd[i])
                        wca_sb = wpoolA.tile([HD, 9, HD], BF16, tag="wca")
                        nc.gpsimd.dma_start(out=wca_sb, in_=wca[i])
                        wcb_sb = wpoolA.tile([HD, 9, HD], BF16, tag="wcb")
                        nc.gpsimd.dma_start(out=wcb_sb, in_=wcb[i])
                        wu_sb = wpoolA.tile([HD, HD], BF16, tag="wu")
                        nc.gpsimd.dma_start(out=wu_sb, in_=wu[i])
                        wv_sb = wpoolA.tile([HD, HD], BF16, tag="wv")
                        nc.gpsimd.dma_start(out=wv_sb, in_=wv[i])
                        nc.gpsimd.dma_start(out=xp_sb[:, c1:c2],
                                            in_=xps[i][:, c1:c2])
                        nc.sync.dma_start(out=xp_sb[:, c2:gm["BUF"]],
                                          in_=xps[i][:, c2:])
                    if i == 1:
                        emit_phaseB_weight_dmas()

                    # max used extent: branch 1 (lo=768, rlen=4608 -> 5376)
                    moe_buf = mpool.tile([HD, 5376], BF16, tag="moe")
                    u_buf = mpool.tile([HD, 5376], BF16, tag="u")

                    st = {}

                    def conv_a(g):
                        fo, n = grs[g]
                        plg = psC.tile([HD, NT], F32, tag="ps", name="plg")
                        nc.tensor.matmul(plg[:, :n], wgd_sb,
                                         xp_sb[:, fo:fo + n],
                                         start=True, stop=True)
                        ex = gpool.tile([HD, NT], BF16, tag="ex")
                        nc.scalar.activation(ex[:, :n], plg[:, :n],
                                             mybir.ActivationFunctionType.Tanh,
                                             scale=-0.5)
                        pa = psC.tile([HD, NT], F32, tag="ps", name="pa")
                        for ti, (dr, ds) in enumerate(TAPS_A[i]):
                            o = dr * gm["SP"] + ds
                            nc.tensor.matmul(pa[:, :n], wca_sb[:, ti, :],
                                             xp_sb[:, fo + o: fo + o + n],
                                             start=(ti == 0), stop=(ti == 8))
                        st[g] = (ex, pa)

                    def conv_b(g):
                        fo, n = grs[g]
                        pb = psC.tile([HD, NT], F32, tag="ps", name="pb")
                        for ti, (dr, ds) in enumerate(TAPS_B[i]):
                            o = dr * gm["SP"] + ds
                            nc.tensor.matmul(pb[:, :n], wcb_sb[:, ti, :],
                                             xp_sb[:, fo + o: fo + o + n],
                                             start=(ti == 0), stop=(ti == 8))
                        st[g] = st[g] + (pb,)

                    def moe_math(g):
                        fo, n = grs[g]
                        ex, pa, pb = st.pop(g)
                        ca = gpool.tile([HD, NT], BF16, tag="ca")
                        nc.scalar.activation(ca[:, :n], pa[:, :n],
                                             mybir.ActivationFunctionType.Identity,
                                             bias=bca_sb[:, i:i + 1], scale=0.5)
                        cb = gpool.tile([HD, NT], BF16, tag="cb")
                        nc.scalar.activation(cb[:, :n], pb[:, :n],
                                             mybir.ActivationFunctionType.Identity,
                                             bias=bcb_sb[:, i:i + 1], scale=0.5)
                        dd = gpool.tile([HD, NT], BF16, tag="dd")
                        nc.vector.tensor_sub(dd[:, :n], ca[:, :n], cb[:, :n])
                        d2 = gpool.tile([HD, NT], BF16, tag="d2")
                        nc.vector.tensor_mul(d2[:, :n], dd[:, :n], ex[:, :n])
                        ss = gpool.tile([HD, NT], BF16, tag="ss")
                        nc.vector.tensor_add(ss[:, :n], ca[:, :n], cb[:, :n])
                        nc.vector.tensor_add(moe_buf[:, fo:fo + n],
                                             ss[:, :n], d2[:, :n])

                    def qk(g):
                        fo, n = grs[g]
                        pq = psT.tile([HD, NT], F32, tag="ps", name="pq")
                        nc.tensor.matmul(pq[:, :n], wu_sb,
                                         moe_buf[:, fo:fo + n],
                                         start=True, stop=True)
                        nc.scalar.copy(u_buf[:, fo:fo + n], pq[:, :n])

                    def attn_s1(a, pspool=psT, pscpool=None, psctag="ps"):
                        if pscpool is None:
                            pscpool = pspool
                        offs = [_row_off(i, 4 * a + j) for j in range(4)]
                        pvt = pspool.tile([96, 4 * HD], F32, tag="ps", name="pvt")
                        for j in range(4):
                            nc.tensor.matmul(pvt[:, j * HD:(j + 1) * HD],
                                             moe_buf[:, offs[j]:offs[j] + 96],
                                             wv_sb, start=True, stop=True)
                        vt = apool.tile([96, 4 * HD], BF16, tag="vt")
                        nc.vector.tensor_copy(vt, pvt)
                        psc = pscpool.tile([96, GN], F32, tag=psctag,
                                           name="psc")
                        for j in range(4):
                            nc.tensor.matmul(psc[:, j * 96:(j + 1) * 96],
                                             u_buf[:, offs[j]:offs[j] + 96],
                                             moe_buf[:, offs[j]:offs[j] + 96],
                                             start=True, stop=True)
                        probs = apool.tile([96, GN], BF16, tag="probs")
                        nc.scalar.activation(probs, psc,
                                             mybir.ActivationFunctionType.Exp,
                                             scale=SCALE)
                        zsum = apool.tile([96, 4], F32, tag="zsum")
                        nc.vector.tensor_reduce(
                            zsum, probs.rearrange("p (j q) -> p j q", q=96),
                            axis=mybir.AxisListType.X, op=mybir.AluOpType.add)
                        rec = apool.tile([96, 4], F32, tag="rec")
                        nc.vector.reciprocal(rec, zsum)
                        pn = apool.tile([96, GN], BF16, tag="pn")
                        for j in range(4):
                            nc.vector.tensor_scalar_mul(
                                pn[:, j * 96:(j + 1) * 96],
                                probs[:, j * 96:(j + 1) * 96],
                                rec[:, j:j + 1])
                        return (a, vt, pn)

                    def attn_s2(s, pspool=psT, ptag="ps", i=i):
                        a, vt, pn = s
                        ppt = pspool.tile([96, GN], BF16, tag=ptag, name="ppt")
                        for j in range(4):
                            nc.tensor.transpose(ppt[:, j * 96:(j + 1) * 96],
                                                pn[:, j * 96:(j + 1) * 96],
                                                identb[:96, :96])
                        pt = apool.tile([96, GN], BF16, tag="pt")
                        nc.vector.tensor_copy(pt, ppt)
                        po = pspool.tile([HD, GN], F32, tag=ptag, name="po")
                        for j in range(4):
                            nc.tensor.matmul(po[:, j * 96:(j + 1) * 96],
                                             vt[:, j * HD:(j + 1) * HD],
                                             pt[:, j * 96:(j + 1) * 96],
                                             start=True, stop=True)
                        nc.scalar.copy(xc_t[i][:, a * GN:(a + 1) * GN], po)

                    conv_a(0)
                    conv_b(0)
                    a_next = 0
                    done = 0
                    for g in range(G):
                        if g + 1 < G:
                            conv_a(g + 1)
                        # drain at most one pending s2 here; the rest after
                        # conv_b so the softmax vector chain has more slack
                        if len(pend_s2) > 1:
                            bi, f2, s2 = pend_s2.pop(0)
                            f2(s2)
                            if bi == 2:
                                s2done[0] += 1
                        # tiles 0/1's gating hoisted under branch 2's conv
                        # stream as soon as the needed xc columns exist, so
                        # phase B starts with gates two tiles ahead
                        if i == 2 and s2done[0] >= 3 and gstage[0] == 0:
                            gstage[0] = 1
                            tail_lsb0 = gating_part1a(0, psT)
                            tail_gm0 = gating_part1c(
                                gating_part1b(tail_lsb0, psT))
                        elif i == 2 and s2done[0] >= 4 and gstage[0] == 1:
                            gstage[0] = 2
                            tail_g0 = gating_part2(tail_gm0, psT)
                            tail_pgbs0 = [emit_pgb_e(tail_g0, e, psT, "ps", 0)
                                          for e in range(3)]
                            tail_lsb1 = gating_part1a(1, psT)
                        elif i == 2 and s2done[0] >= 5 and gstage[0] == 2:
                            gstage[0] = 3
                            tail_g1 = gating_part2(gating_part1c(
                                gating_part1b(tail_lsb1, psT)), psT)
                        moe_math(g)
                        if g + 1 < G:
                            conv_b(g + 1)
                        while len(pend_s2) > 1:
                            bi, f2, s2 = pend_s2.pop(0)
                            f2(s2)
                            if bi == 2:
                                s2done[0] += 1
                        qk(g)
                        done += grs[g][1]
                        cov = gm["lo"] + done
                        amax = 6 if i == 2 else 12
                        while a_next < amax and \
                                _row_off(i, 4 * a_next + 3) + 96 <= cov:
                            pend_s2.append((i, attn_s2, attn_s1(a_next)))
                            a_next += 1
                    # branch tail s2's carry into the next branch's conv
                    # stream; branch 2 drains fully before the phase-B tail.
                    if i == 2:
                        while pend_s2:
                            bi, f2, s2 = pend_s2.pop(0)
                            f2(s2)
                            if bi == 2:
                                s2done[0] += 1
                        for a in range(6, 12):
                            deferred.append((attn_s1, attn_s2, a))
                # fallback if the hoist conditions never fired late enough
                if gstage[0] < 3:
                    if gstage[0] == 0:
                        tail_lsb0 = gating_part1a(0, psT)
                        tail_gm0 = gating_part1c(
                            gating_part1b(tail_lsb0, psT))
                        gstage[0] = 1
                    if gstage[0] == 1:
                        tail_g0 = gating_part2(tail_gm0, psT)
                        tail_pgbs0 = [emit_pgb_e(tail_g0, e, psT, "ps", 0)
                                      for e in range(3)]
                        tail_lsb1 = gating_part1a(1, psT)
                        gstage[0] = 2
                    tail_g1 = gating_part2(gating_part1c(
                        gating_part1b(tail_lsb1, psT)), psT)
                    gstage[0] = 3
                g0state[0] = (tail_g0, tail_pgbs0, tail_g1)
            stackX.close()

            # ---------------- Phase B: final MLP MoE + proj ---------------
            with tc.tile_pool(name="bpool", bufs=3) as bpool, \
                 tc.tile_pool(name="hpool", bufs=5) as hpool, \
                 tc.tile_pool(name="psL", bufs=3, space="PSUM") as psL, \
                 tc.tile_pool(name="psGB", bufs=1, space="PSUM") as psGB, \
                 tc.tile_pool(name="psPG", bufs=1, space="PSUM") as psPG, \
                 tc.tile_pool(name="psB", bufs=3, space="PSUM") as psB:
                w1_sb = wB["w1"]
                w2_sb = wB["w2"]
                b1_sb = wB["b1"]
                b2r_sb = wB["b2r"]

                LA = 4
                iters = [(e, m) for e in range(3) for m in range(12)]
                gates_cur, pgbs_cur, gates_next = g0state[0]
                lsb_n2 = None
                gates_n2 = None
                dpend = []
                # deferred branch-2 attention spread over tiles 0-2
                DEF_SCHED = {(0, 4): (1, 0), (0, 14): (2, 0),
                             (0, 20): (1, 1), (0, 29): (2, 1),
                             (1, 4): (1, 2), (1, 14): (2, 2),
                             (1, 20): (1, 3), (1, 29): (2, 3),
                             (2, 4): (1, 4), (2, 14): (2, 4),
                             (2, 20): (1, 5), (2, 29): (2, 5)}
                drain_prev = [None]

                def emit_drain(final=False):
                    if drain_prev[0] is None:
                        return
                    pdp, tp0 = drain_prev[0]
                    drain_prev[0] = None
                    if final:
                        # bias folded into b2r: plain copies fanned across
                        # engines, DMAs fanned across three queues
                        osb0 = bpool.tile([HD, NT], F32, tag="osb")
                        nc.scalar.copy(osb0, pdp[0])
                        nc.sync.dma_start(out=out_cm[0:HD, tp0:tp0 + NT],
                                          in_=osb0)
                        osb1 = bpool.tile([HD, NT], F32, tag="osb")
                        nc.vector.tensor_copy(osb1, pdp[1])
                        nc.scalar.dma_start(
                            out=out_cm[HD:2 * HD, tp0:tp0 + NT], in_=osb1)
                        osb2 = bpool.tile([HD, NT], F32, tag="osb")
                        nc.vector.tensor_copy(osb2, pdp[2])
                        nc.gpsimd.dma_start(
                            out=out_cm[2 * HD:3 * HD, tp0:tp0 + NT], in_=osb2)
                        return
                    for mp in range(3):
                        osb = bpool.tile([HD, NT], F32, tag="osb")
                        nc.scalar.copy(osb, pdp[mp])
                        nc.sync.dma_start(
                            out=out_cm[mp * HD:(mp + 1) * HD, tp0:tp0 + NT],
                            in_=osb)

                for t in range(NTILES):
                    t0 = t * NT
                    pd = [psL.tile([HD, NT], F32, tag="down", name=f"pd{_i}")
                          for _i in range(3)]
                    hs_l = {}
                    for k in range(36 + LA):
                        if k < 36:
                            e, m = iters[k]
                            pu = psB.tile([HD, NT], F32, tag="ps", name="pu")
                            for kc in range(3):
                                nc.tensor.matmul(
                                    pu, w1_sb[e][:, kc, m * HD:(m + 1) * HD],
                                    xc_t[kc][:, t0:t0 + NT],
                                    start=(kc == 0), stop=(kc == 2))
                            h = hpool.tile([HD, NT], BF16, tag="h")
                            nc.scalar.activation(
                                h, pu, mybir.ActivationFunctionType.Gelu,
                                bias=b1_sb[:, e, m:m + 1])
                            hs = hpool.tile([HD, NT], BF16, tag="hs")
                            nc.vector.tensor_mul(hs, h, pgbs_cur[e])
                            hs_l[k] = (e, m, hs)
                            if k == 1:
                                emit_drain()
                            if (t, k) in DEF_SCHED:
                                which, idx = DEF_SCHED[(t, k)]
                                s1f, s2f, a = deferred[idx]
                                if which == 1:
                                    dpend.append(s1f(a, psGB, psPG, "pgb"))
                                else:
                                    s2f(dpend.pop(0), psPG, "pgb")
                            if (e, m) == (0, 0) and t + 2 < NTILES:
                                lsb_n2 = gating_part1a(t + 2, psGB)
                            if (e, m) == (0, 6) and t + 2 < NTILES:
                                st_n2 = gating_part1b(lsb_n2, psGB)
                            if (e, m) == (0, 9) and t + 2 < NTILES:
                                gm_n2 = gating_part1c(st_n2)
                            if (e, m) == (1, 0) and t + 2 < NTILES:
                                gates_n2 = gating_part2(gm_n2, psGB)
                            if e == 2 and m in (7, 9, 11) and t + 1 < NTILES:
                                e_ = (m - 7) // 2
                                pb = emit_pgb_e(gates_next, e_, psPG, "pgb",
                                                t + 1)
                                if e_ == 0:
                                    pgbs_next = []
                                pgbs_next.append(pb)
                        if k >= LA:
                            e2, m2, hs2 = hs_l.pop(k - LA)
                            for mp in range(3):
                                nc.tensor.matmul(
                                    pd[mp],
                                    w2_sb[e2][:, m2, mp * HD:(mp + 1) * HD],
                                    hs2, start=(e2 == 0 and m2 == 0),
                                    stop=False)
                    for mp in range(3):
                        nc.tensor.matmul(pd[mp],
                                         b2r_sb[:, mp * HD:(mp + 1) * HD],
                                         gates_cur, start=False, stop=True)
                    drain_prev[0] = (pd, t0)
                    if t + 1 < NTILES:
                        gates_cur, pgbs_cur = gates_next, pgbs_next
                        gates_next = gates_n2
                emit_drain(final=True)
            stackA.close()
    nc.compile()
    return nc


def _prep_inputs(x, w_e1, b_e1, w_e2, b_e2, w_e3, b_e3, w_e4, b_e4, w_e5, b_e5,
                 w_e6, b_e6, wg1, wg2, wg3, w_qkv, w_attn_proj, b_attn_proj,
                 wg_final, w_mlp1, b_mlp1, w_mlp2, b_mlp2, w_proj, b_proj):
    f = np.float32
    shared = {}
    shared["wca"] = np.ascontiguousarray(np.stack([
        w_e1.reshape(9, HD, HD), w_e3.reshape(9, HD, HD),
        w_e5.reshape(9, HD, HD)]).transpose(0, 2, 1, 3), dtype=f).astype(BFNP)
    shared["wcb"] = np.ascontiguousarray(np.stack([
        w_e2.reshape(9, HD, HD), w_e4.reshape(9, HD, HD),
        w_e6.reshape(9, HD, HD)]).transpose(0, 2, 1, 3), dtype=f).astype(BFNP)
    shared["bca"] = np.ascontiguousarray(
        np.stack([b_e1, b_e3, b_e5], axis=1) * 0.5, dtype=f)
    shared["bcb"] = np.ascontiguousarray(
        np.stack([b_e2, b_e4, b_e6], axis=1) * 0.5, dtype=f)
    wgs = np.stack([wg1, wg2, wg3])
    shared["wgd"] = np.ascontiguousarray(
        np.repeat((wgs[:, :, 1] - wgs[:, :, 0])[:, :, None], HD, axis=2),
        dtype=f).astype(BFNP)
    eb3 = np.zeros((3, 384), f)
    for e in range(3):
        eb3[e, e * 128:(e + 1) * 128] = 1.0
    shared["eb3"] = eb3.astype(BFNP)
    wq64 = np.asarray(w_qkv[:, :, :HD], dtype=np.float64)
    wk64 = np.asarray(w_qkv[:, :, HD:256], dtype=np.float64)
    shared["wu"] = np.ascontiguousarray(
        np.einsum("icq,idq->icd", wq64, wk64), dtype=f).astype(BFNP)
    wv64 = np.asarray(w_qkv[:, :, 256:], dtype=np.float64)
    wap64 = np.asarray(w_attn_proj, dtype=np.float64)
    shared["wv"] = np.ascontiguousarray(
        np.einsum("ick,iko->ico", wv64, wap64), dtype=f).astype(BFNP)
    # fold the attention-proj bias into the MLP/gate paths (xc on device
    # is stored without it): b1' = b1 + bap @ w1; lgb = bap @ wg_final
    bap64 = np.asarray(b_attn_proj, np.float64).reshape(C)
    shared["lgb"] = np.tile(
        (bap64 @ np.asarray(wg_final, np.float64)).reshape(3, 1),
        (1, 4)).astype(f)
    shared["wgf"] = np.ascontiguousarray(
        np.tile(wg_final.reshape(3, HD, 3), (1, 1, 43))[:, :, :HD],
        dtype=f).astype(BFNP)
    shared["w1"] = np.ascontiguousarray(
        w_mlp1.reshape(3, 3, HD, 1536).transpose(0, 2, 1, 3),
        dtype=f).astype(BFNP)
    b1p = np.asarray(b_mlp1, np.float64) + \
        np.einsum("c,ecf->ef", bap64, np.asarray(w_mlp1, np.float64))
    shared["b1"] = np.ascontiguousarray(
        b1p.reshape(3, 12, HD).transpose(2, 0, 1), dtype=f)
    w2p = np.asarray(w_mlp2, dtype=np.float64) @ np.asarray(w_proj, np.float64)
    shared["w2"] = np.ascontiguousarray(
        w2p.reshape(3, 12, HD, C).transpose(0, 2, 1, 3), dtype=f).astype(BFNP)
    # top-2 gates sum to exactly 1, so the final projection bias folds
    # into the gate-weighted b2r term: sum_e g_e (b2r_e + bpr) adds bpr once
    shared["b2r"] = np.ascontiguousarray(
        np.asarray(b_mlp2, np.float64) @ np.asarray(w_proj, np.float64)
        + np.asarray(b_proj, np.float64)[None, :],
        dtype=f).astype(BFNP)

    in_maps = []
    xf = np.asarray(x, dtype=f)
    for c in range(N_CORES):
        b, halfc = c // 2, c % 2
        r0 = halfc * R
        m = dict(shared)
        for i in range(3):
            g = GEOM[i]
            xi = xf[b, :, :, i * HD:(i + 1) * HD]  # [96, 96, 128]
            plane = np.zeros((HD, g["NR"], g["SP"]), f)
            glo = max(0, r0 - g["pad_r"])
            ghi = min(HH, r0 + R + g["pad_r"])
            plo = glo - (r0 - g["pad_r"])
            plane[:, plo:plo + (ghi - glo),
                  g["pad_c"]:g["pad_c"] + 96] = \
                xi[glo:ghi].transpose(2, 0, 1)
            buf = np.zeros((HD, g["BUF"]), f)
            buf[:, g["OFF"]:g["OFF"] + g["NR"] * g["SP"]] = \
                plane.reshape(HD, -1)
            m[f"xp{i}"] = buf.astype(BFNP)
        m["xp2"][:, GEOM[2]["OFF"] + GEOM[2]["NR"] * GEOM[2]["SP"]:] = 0
        in_maps.append(m)
    return in_maps


def kernel(**inputs):
    global _CACHED_NC
    if _CACHED_NC is None:
        _CACHED_NC = build_kernel()
    nc = _CACHED_NC
    in_maps = _prep_inputs(**{k: np.asarray(v) for k, v in inputs.items()})
    res = None
    for attempt in range(3):
        try:
            res = run_bass_kernel_spmd(nc, in_maps,
                                       core_ids=list(range(N_CORES)))
            break
        except Exception:
            if attempt == 2:
                raise
            import time
            time.sleep(2.0)
    out = np.empty((B, HH, WW, C), np.float32)
    for c in range(N_CORES):
        b, halfc = c // 2, c % 2
        slab = res.results[c]["out_cm"].reshape(C, R, 96)
        out[b, :, halfc * R:(halfc + 1) * R, :] = slab.transpose(2, 1, 0)
    return out



# revision 72
# speedup vs baseline: 1.0067x; 1.0032x over previous
"""Trainium2 Bass kernel for nn_MAMoE (conv-MoE -> row attention -> MLP-MoE).

Sharding: 8 cores = (batch b in 0..3) x (H-half in 0..1). All routing is
per-token; the reference's swapaxes(1,2) means attention row r produces
output column w=r, so each core independently computes the full pipeline
for its 48 attention rows and the host reassembles along W.

v3: all matmuls bf16 (f32 PSUM accumulate); per-branch minimal conv
padding (100/96/112 row pitch); two-stage attention emission so the
softmax chain hides under the next conv group's matmuls; software-
pipelined phase B (ups run 2 iterations ahead of downs); first tile's
gating hoisted into the phase A tail.
"""
import contextlib

import numpy as np
import ml_dtypes

import concourse.bass as bass
import concourse.bass_isa as bass_isa
import concourse.mybir as mybir
import concourse.tile as tile
from concourse import bacc
from concourse.bass_utils import run_bass_kernel_spmd
from concourse.masks import make_identity

F32 = mybir.dt.float32
BF16 = mybir.dt.bfloat16
BFNP = ml_dtypes.bfloat16

B, HH, WW, C = 4, 96, 96, 384
HD = 128
SCALE = float((HD // 3) ** -0.5)  # 42**-0.5
N_CORES = 8
R = 48            # attention rows per core
T = R * 96        # tokens per core = 4608
NT = 512          # tokens per MLP tile
NTILES = T // NT  # 9
GN = 4 * 96       # tokens per attention group = 384

# per-branch padded-plane geometry
GEOM = [
    dict(SP=100, NR=52, OFF=8, BUF=5248, pad_r=2, pad_c=2),   # 3x3 convs
    dict(SP=96, NR=64, OFF=0, BUF=6144, pad_r=8, pad_c=0),    # (9,1) convs
    dict(SP=104, NR=48, OFF=8, BUF=5120, pad_r=0, pad_c=0),   # (1,9) convs
]
for _g in GEOM:
    _g["lo"] = _g["OFF"] + _g["pad_r"] * _g["SP"]
    _g["rlen"] = 48 * _g["SP"]
GEOM[2]["rlen"] = 47 * 104 + 96

TAPS_A = [
    [(dr, ds) for dr in (-1, 0, 1) for ds in (-1, 0, 1)],
    [(dr, 0) for dr in range(-4, 5)],
    [(0, ds) for ds in range(-4, 5)],
]
TAPS_B = [
    [(dr, ds) for dr in (-2, 0, 2) for ds in (-2, 0, 2)],
    [(dr, 0) for dr in range(-8, 9, 2)],
    [(0, ds) for ds in range(-8, 9, 2)],
]


def _row_off(i, r):
    g = GEOM[i]
    return g["OFF"] + (g["pad_r"] + r) * g["SP"] + g["pad_c"]


def _groups(i):
    g = GEOM[i]
    out = []
    fo = g["lo"]
    end = g["lo"] + g["rlen"]
    while fo < end:
        out.append((fo, min(NT, end - fo)))
        fo += NT
    return out


_CACHED_NC = None


def build_kernel():
    nc = bacc.Bacc("TRN2", target_bir_lowering=False, debug=False)

    xps = [nc.dram_tensor(f"xp{i}", [HD, GEOM[i]["BUF"]], BF16,
                          kind="ExternalInput").ap() for i in range(3)]
    wca = nc.dram_tensor("wca", [3, HD, 9, HD], BF16, kind="ExternalInput").ap()
    wcb = nc.dram_tensor("wcb", [3, HD, 9, HD], BF16, kind="ExternalInput").ap()
    bca = nc.dram_tensor("bca", [HD, 3], F32, kind="ExternalInput").ap()
    bcb = nc.dram_tensor("bcb", [HD, 3], F32, kind="ExternalInput").ap()
    wgd = nc.dram_tensor("wgd", [3, HD, HD], BF16, kind="ExternalInput").ap()
    eb3 = nc.dram_tensor("eb3", [3, 384], BF16, kind="ExternalInput").ap()
    wu = nc.dram_tensor("wu", [3, HD, HD], BF16, kind="ExternalInput").ap()
    wv = nc.dram_tensor("wv", [3, HD, HD], BF16, kind="ExternalInput").ap()
    lgb = nc.dram_tensor("lgb", [3, 4], F32, kind="ExternalInput").ap()
    wgf = nc.dram_tensor("wgf", [3, HD, HD], BF16, kind="ExternalInput").ap()
    w1 = nc.dram_tensor("w1", [3, HD, 3, 1536], BF16, kind="ExternalInput").ap()
    b1 = nc.dram_tensor("b1", [HD, 3, 12], F32, kind="ExternalInput").ap()
    w2 = nc.dram_tensor("w2", [3, HD, 12, C], BF16, kind="ExternalInput").ap()
    b2r = nc.dram_tensor("b2r", [3, C], BF16, kind="ExternalInput").ap()
    bpr = nc.dram_tensor("bpr", [HD, 3], F32, kind="ExternalInput").ap()
    out_cm = nc.dram_tensor("out_cm", [C, T], F32, kind="ExternalOutput").ap()

    with tile.TileContext(nc) as tc:
        with tc.tile_pool(name="persist", bufs=1) as persist, \
             tc.tile_pool(name="wpoolB", bufs=1) as wpoolB, \
             tc.tile_pool(name="gbpool", bufs=2) as gbpool, \
             tc.tile_pool(name="gpoolB", bufs=2) as gpoolB:
            stackA = contextlib.ExitStack()
            mpool = stackA.enter_context(tc.tile_pool(name="mpool", bufs=2))
            wpoolA = stackA.enter_context(tc.tile_pool(name="wpoolA", bufs=2))
            apool = stackA.enter_context(tc.tile_pool(name="apool", bufs=4))
            anorm = stackA.enter_context(tc.tile_pool(name="anorm", bufs=2))
            stackX = contextlib.ExitStack()
            xpool = stackX.enter_context(tc.tile_pool(name="xpool", bufs=2))
            # ---- branch-0 critical DMAs first (minimize first-matmul wait);
            # spread the first transfers across 4 queues so their fixed DMA
            # latencies overlap.
            xp_sb0 = xpool.tile([HD, 6144], BF16, tag="xp")
            c1, c2 = 1024, 3072
            nc.sync.dma_start(out=xp_sb0[:, :c1], in_=xps[0][:, :c1])
            wgd_sb0 = wpoolA.tile([HD, HD], BF16, tag="wgd")
            nc.scalar.dma_start(out=wgd_sb0, in_=wgd[0])
            wca_sb0 = wpoolA.tile([HD, 9, HD], BF16, tag="wca")
            nc.gpsimd.dma_start(out=wca_sb0, in_=wca[0])
            wcb_sb0 = wpoolA.tile([HD, 9, HD], BF16, tag="wcb")
            nc.gpsimd.dma_start(out=wcb_sb0, in_=wcb[0])
            wu_sb0 = wpoolA.tile([HD, HD], BF16, tag="wu")
            nc.gpsimd.dma_start(out=wu_sb0, in_=wu[0])
            wv_sb0 = wpoolA.tile([HD, HD], BF16, tag="wv")
            nc.gpsimd.dma_start(out=wv_sb0, in_=wv[0])
            nc.gpsimd.dma_start(out=xp_sb0[:, c1:c2], in_=xps[0][:, c1:c2])
            nc.sync.dma_start(out=xp_sb0[:, c2:GEOM[0]["BUF"]],
                              in_=xps[0][:, c2:])
            bca_sb = persist.tile([HD, 3], F32)
            bcb_sb = persist.tile([HD, 3], F32)
            lgb_sb = persist.tile([3, 4], F32)
            nc.sync.dma_start(out=bca_sb, in_=bca)
            nc.sync.dma_start(out=bcb_sb, in_=bcb)
            nc.sync.dma_start(out=lgb_sb, in_=lgb)

            br0 = dict(xp=xp_sb0, wgd=wgd_sb0, wca=wca_sb0, wcb=wcb_sb0,
                       wu=wu_sb0, wv=wv_sb0)

            identb = persist.tile([HD, HD], BF16)
            make_identity(nc, identb)

            xc_t = [persist.tile([HD, T], BF16, tag=f"xc{i}", name=f"xc{i}")
                    for i in range(3)]

            wB = {}

            def emit_phaseB_weight_dmas():
                wB["b1"] = wpoolB.tile([HD, 3, 12], F32, tag="b1", name="b1s")
                nc.sync.dma_start(out=wB["b1"], in_=b1)
                wB["b2r"] = wpoolB.tile([3, C], BF16, tag="b2r", name="b2rs")
                nc.sync.dma_start(out=wB["b2r"], in_=b2r)
                wB["wgf"] = wpoolB.tile([HD, 3, HD], BF16, tag="wgf", name="wgfs")
                nc.sync.dma_start(out=wB["wgf"],
                                  in_=wgf.rearrange("a p b -> p a b"))
                wB["bpr"] = wpoolB.tile([HD, 3], F32, tag="bpr", name="bprs")
                nc.sync.dma_start(out=wB["bpr"], in_=bpr)
                wB["eb3"] = wpoolB.tile([3, 384], BF16, tag="eb3", name="eb3s")
                nc.sync.dma_start(out=wB["eb3"], in_=eb3)
                wB["w1"] = []
                wB["w2"] = []
                for e in range(3):
                    t1 = wpoolB.tile([HD, 3, 1536], BF16, tag=f"w1_{e}",
                                     name=f"w1_{e}")
                    nc.sync.dma_start(out=t1, in_=w1[e])
                    wB["w1"].append(t1)
                    t2 = wpoolB.tile([HD, 12, C], BF16, tag=f"w2_{e}",
                                     name=f"w2_{e}")
                    nc.sync.dma_start(out=t2, in_=w2[e])
                    wB["w2"].append(t2)

            def gating_part1a(t, pool):
                """logits matmul for tile t."""
                t0 = t * NT
                plg = pool.tile([HD, NT], F32, tag="ps", name="plg")
                for kc in range(3):
                    nc.tensor.matmul(plg, wB["wgf"][:, kc, :],
                                     xc_t[kc][:, t0:t0 + NT],
                                     start=(kc == 0), stop=(kc == 2))
                lsb = gpoolB.tile([3, NT], BF16, tag="lsb", name="lsb")
                nc.scalar.activation(lsb, plg[0:3, :],
                                     mybir.ActivationFunctionType.Identity,
                                     bias=lgb_sb[:, 0:1])
                return lsb

            def gating_part1b(lsb, pool):
                """token-major top-2 softmax math, first half."""
                pltf = pool.tile([HD, NT], BF16, tag="ps", name="plt")
                # 4-col stride keeps each bf16 PSUM write 4-byte aligned
                for t4 in range(4):
                    nc.tensor.transpose(pltf[:, t4 * 4:t4 * 4 + 3],
                                        lsb[:, t4 * HD:(t4 + 1) * HD],
                                        identb[:3, :3])
                lt = gpoolB.tile([HD, 12], F32, tag="lt", name="lt")
                nc.vector.tensor_copy(
                    lt.rearrange("p (g c) -> p g c", c=3),
                    pltf[:, :16].rearrange("p (g c) -> p g c", c=4)[:, :, 0:3])
                l3 = lt.rearrange("p (j e) -> p j e", e=3)
                mx = gpoolB.tile([HD, 4], F32, tag="mx", name="mx")
                nc.vector.tensor_reduce(mx, l3, axis=mybir.AxisListType.X,
                                        op=mybir.AluOpType.max)
                mn = gpoolB.tile([HD, 4], F32, tag="mn", name="mn")
                nc.vector.tensor_reduce(mn, l3, axis=mybir.AxisListType.X,
                                        op=mybir.AluOpType.min)
                sm = gpoolB.tile([HD, 4], F32, tag="sm", name="sm")
                nc.vector.tensor_reduce(sm, l3, axis=mybir.AxisListType.X,
                                        op=mybir.AluOpType.add)
                t1 = gpoolB.tile([HD, 4], F32, tag="t1", name="t1")
                nc.vector.tensor_sub(t1, sm, mx)
                mid = gpoolB.tile([HD, 4], F32, tag="mid", name="mid")
                nc.vector.tensor_sub(mid, t1, mn)
                dm = gpoolB.tile([HD, 4], F32, tag="dm", name="dm")
                nc.vector.tensor_sub(dm, mx, mid)
                th = gpoolB.tile([HD, 4], F32, tag="th", name="th")
                nc.scalar.activation(th, dm,
                                     mybir.ActivationFunctionType.Tanh,
                                     scale=0.5)
                return (lt, mx, mn, th)

            def gating_part1c(st):
                """token-major top-2 softmax math, second half."""
                lt, mx, mn, th = st
                gmx = gpoolB.tile([HD, 4], F32, tag="gmx", name="gmx")
                nc.vector.tensor_scalar(gmx, th, 0.5, 0.5,
                                        op0=mybir.AluOpType.mult,
                                        op1=mybir.AluOpType.add)
                eqx = gpoolB.tile([HD, 12], F32, tag="eqx", name="eqx")
                eqn = gpoolB.tile([HD, 12], F32, tag="eqn", name="eqn")
                for t4 in range(4):
                    sl = slice(t4 * 3, (t4 + 1) * 3)
                    nc.vector.tensor_scalar(eqx[:, sl], lt[:, sl],
                                            mx[:, t4:t4 + 1], None,
                                            op0=mybir.AluOpType.is_equal)
                    nc.vector.tensor_scalar(eqn[:, sl], lt[:, sl],
                                            mn[:, t4:t4 + 1], None,
                                            op0=mybir.AluOpType.is_equal)
                # u = 1 - eqx - eqn (mid indicator); g = gmx*(eqx-u) + u
                s1 = gpoolB.tile([HD, 12], F32, tag="s1", name="s1")
                nc.vector.tensor_add(s1, eqx, eqn)
                u = gpoolB.tile([HD, 12], F32, tag="u", name="u")
                nc.vector.tensor_scalar(u, s1, -1.0, 1.0,
                                        op0=mybir.AluOpType.mult,
                                        op1=mybir.AluOpType.add)
                d0 = gpoolB.tile([HD, 12], F32, tag="d0", name="d0")
                nc.vector.tensor_sub(d0, eqx, u)
                p0 = gpoolB.tile([HD, 12], F32, tag="p0", name="p0")
                for t4 in range(4):
                    sl = slice(t4 * 3, (t4 + 1) * 3)
                    nc.vector.tensor_scalar_mul(p0[:, sl], d0[:, sl],
                                                gmx[:, t4:t4 + 1])
                gm2 = gpoolB.tile([HD, 12], BF16, tag="gm", name="gm")
                nc.vector.tensor_add(gm2, p0, u)
                return gm2

            def gating_part2(gm2, pool):
                """expert-major gates [3, NT] from token-major gm."""
                pgtf = pool.tile([HD, NT], BF16, tag="ps", name="pgt")
                pgt = pgtf[0:3, :]
                for t4 in range(4):
                    nc.tensor.transpose(pgt[:, t4 * HD:(t4 + 1) * HD],
                                        gm2[:, t4 * 3:(t4 + 1) * 3],
                                        identb)
                gates_r = gpoolB.tile([3, NT], BF16, tag="gates",
                                      name="gates_r", bufs=3)
                nc.scalar.copy(gates_r, pgt)
                return gates_r

            def emit_pgb_e(gates_r, e, pspool, tag, t):
                """gate row e broadcast to 128 partitions via one-hot MM."""
                pgb = pspool.tile([HD, NT], F32, tag=tag, name="pgbp")
                nc.tensor.matmul(pgb, wB["eb3"][:, e * HD:(e + 1) * HD],
                                 gates_r, start=True, stop=True)
                pb = gbpool.tile([HD, NT], BF16, tag=f"pgb{e}",
                                 name=f"pgb{e}_{t}")
                nc.scalar.copy(pb, pgb)
                return pb

            g0state = [None]

            # ------------ Phase A: conv MoE + attention per branch --------
            deferred = []
            pend_s2 = []
            s2done = [0]
            gstage = [0]
            with tc.tile_pool(name="gpool", bufs=2) as gpool, \
                 tc.tile_pool(name="psC", bufs=4, space="PSUM") as psC, \
                 tc.tile_pool(name="psT", bufs=4, space="PSUM") as psT:
                for i in range(3):
                    gm = GEOM[i]
                    grs = _groups(i)
                    G = len(grs)
                    if i == 0:
                        xp_sb = br0["xp"]
                        wgd_sb, wca_sb, wcb_sb = \
                            br0["wgd"], br0["wca"], br0["wcb"]
                        wu_sb, wv_sb = br0["wu"], br0["wv"]
                    else:
                        xp_sb = xpool.tile([HD, 6144], BF16, tag="xp")
                        nc.gpsimd.dma_start(out=xp_sb[:, :c1],
                                            in_=xps[i][:, :c1])
                        wgd_sb = wpoolA.tile([HD, HD], BF16, tag="wgd")
                        nc.gpsimd.dma_start(out=wgd_sb, in_=wgd[i])
                        wca_sb = wpoolA.tile([HD, 9, HD], BF16, tag="wca")
                        nc.gpsimd.dma_start(out=wca_sb, in_=wca[i])
                        wcb_sb = wpoolA.tile([HD, 9, HD], BF16, tag="wcb")
                        nc.gpsimd.dma_start(out=wcb_sb, in_=wcb[i])
                        wu_sb = wpoolA.tile([HD, HD], BF16, tag="wu")
                        nc.gpsimd.dma_start(out=wu_sb, in_=wu[i])
                        wv_sb = wpoolA.tile([HD, HD], BF16, tag="wv")
                        nc.gpsimd.dma_start(out=wv_sb, in_=wv[i])
                        nc.gpsimd.dma_start(out=xp_sb[:, c1:c2],
                                            in_=xps[i][:, c1:c2])
                        nc.sync.dma_start(out=xp_sb[:, c2:gm["BUF"]],
                                          in_=xps[i][:, c2:])
                    if i == 1:
                        emit_phaseB_weight_dmas()

                    # max used extent: branch 1 (lo=768, rlen=4608 -> 5376)
                    moe_buf = mpool.tile([HD, 5376], BF16, tag="moe")
                    u_buf = mpool.tile([HD, 5376], BF16, tag="u")

                    st = {}

                    def conv_a(g):
                        fo, n = grs[g]
                        plg = psC.tile([HD, NT], F32, tag="ps", name="plg")
                        nc.tensor.matmul(plg[:, :n], wgd_sb,
                                         xp_sb[:, fo:fo + n],
                                         start=True, stop=True)
                        ex = gpool.tile([HD, NT], BF16, tag="ex")
                        nc.scalar.activation(ex[:, :n], plg[:, :n],
                                             mybir.ActivationFunctionType.Tanh,
                                             scale=-0.5)
                        pa = psC.tile([HD, NT], F32, tag="ps", name="pa")
                        for ti, (dr, ds) in enumerate(TAPS_A[i]):
                            o = dr * gm["SP"] + ds
                            nc.tensor.matmul(pa[:, :n], wca_sb[:, ti, :],
                                             xp_sb[:, fo + o: fo + o + n],
                                             start=(ti == 0), stop=(ti == 8))
                        st[g] = (ex, pa)

                    def conv_b(g):
                        fo, n = grs[g]
                        pb = psC.tile([HD, NT], F32, tag="ps", name="pb")
                        for ti, (dr, ds) in enumerate(TAPS_B[i]):
                            o = dr * gm["SP"] + ds
                            nc.tensor.matmul(pb[:, :n], wcb_sb[:, ti, :],
                                             xp_sb[:, fo + o: fo + o + n],
                                             start=(ti == 0), stop=(ti == 8))
                        st[g] = st[g] + (pb,)

                    def moe_math(g):
                        fo, n = grs[g]
                        ex, pa, pb = st.pop(g)
                        ca = gpool.tile([HD, NT], BF16, tag="ca")
                        nc.scalar.activation(ca[:, :n], pa[:, :n],
                                             mybir.ActivationFunctionType.Identity,
                                             bias=bca_sb[:, i:i + 1], scale=0.5)
                        cb = gpool.tile([HD, NT], BF16, tag="cb")
                        nc.scalar.activation(cb[:, :n], pb[:, :n],
                                             mybir.ActivationFunctionType.Identity,
                                             bias=bcb_sb[:, i:i + 1], scale=0.5)
                        dd = gpool.tile([HD, NT], BF16, tag="dd")
                        nc.vector.tensor_sub(dd[:, :n], ca[:, :n], cb[:, :n])
                        d2 = gpool.tile([HD, NT], BF16, tag="d2")
                        nc.vector.tensor_mul(d2[:, :n], dd[:, :n], ex[:, :n])
                        ss = gpool.tile([HD, NT], BF16, tag="ss")
                        nc.vector.tensor_add(ss[:, :n], ca[:, :n], cb[:, :n])
                        nc.vector.tensor_add(moe_buf[:, fo:fo + n],
                                             ss[:, :n], d2[:, :n])

                    def qk(g):
                        fo, n = grs[g]
                        pq = psT.tile([HD, NT], F32, tag="ps", name="pq")
                        nc.tensor.matmul(pq[:, :n], wu_sb,
                                         moe_buf[:, fo:fo + n],
                                         start=True, stop=True)
                        nc.scalar.copy(u_buf[:, fo:fo + n], pq[:, :n])

                    def attn_s1(a, pspool=psT, pscpool=None, psctag="ps"):
                        if pscpool is None:
                            pscpool = pspool
                        offs = [_row_off(i, 4 * a + j) for j in range(4)]
                        pvt = pspool.tile([96, 4 * HD], F32, tag="ps", name="pvt")
                        for j in range(4):
                            nc.tensor.matmul(pvt[:, j * HD:(j + 1) * HD],
                                             moe_buf[:, offs[j]:offs[j] + 96],
                                             wv_sb, start=True, stop=True)
                        vt = apool.tile([96, 4 * HD], BF16, tag="vt")
                        nc.vector.tensor_copy(vt, pvt)
                        psc = pscpool.tile([96, GN], F32, tag=psctag,
                                           name="psc")
                        for j in range(4):
                            nc.tensor.matmul(psc[:, j * 96:(j + 1) * 96],
                                             u_buf[:, offs[j]:offs[j] + 96],
                                             moe_buf[:, offs[j]:offs[j] + 96],
                                             start=True, stop=True)
                        probs = apool.tile([96, GN], BF16, tag="probs")
                        nc.scalar.activation(probs, psc,
                                             mybir.ActivationFunctionType.Exp,
                                             scale=SCALE)
                        zsum = apool.tile([96, 4], F32, tag="zsum")
                        nc.vector.tensor_reduce(
                            zsum, probs.rearrange("p (j q) -> p j q", q=96),
                            axis=mybir.AxisListType.X, op=mybir.AluOpType.add)
                        rec = apool.tile([96, 4], F32, tag="rec")
                        nc.vector.reciprocal(rec, zsum)
                        pn = apool.tile([96, GN], BF16, tag="pn")
                        for j in range(4):
                            nc.vector.tensor_scalar_mul(
                                pn[:, j * 96:(j + 1) * 96],
                                probs[:, j * 96:(j + 1) * 96],
                                rec[:, j:j + 1])
                        return (a, vt, pn)

                    def attn_s2(s, pspool=psT, ptag="ps", i=i):
                        a, vt, pn = s
                        ppt = pspool.tile([96, GN], BF16, tag=ptag, name="ppt")
                        for j in range(4):
                            nc.tensor.transpose(ppt[:, j * 96:(j + 1) * 96],
                                                pn[:, j * 96:(j + 1) * 96],
                                                identb[:96, :96])
                        pt = apool.tile([96, GN], BF16, tag="pt")
                        nc.vector.tensor_copy(pt, ppt)
                        po = pspool.tile([HD, GN], F32, tag=ptag, name="po")
                        for j in range(4):
                            nc.tensor.matmul(po[:, j * 96:(j + 1) * 96],
                                             vt[:, j * HD:(j + 1) * HD],
                                             pt[:, j * 96:(j + 1) * 96],
                                             start=True, stop=True)
                        nc.scalar.copy(xc_t[i][:, a * GN:(a + 1) * GN], po)

                    conv_a(0)
                    conv_b(0)
                    a_next = 0
                    done = 0
                    for g in range(G):
                        if g + 1 < G:
                            conv_a(g + 1)
                        # drain at most one pending s2 here; the rest after
                        # conv_b so the softmax vector chain has more slack
                        if len(pend_s2) > 1:
                            bi, f2, s2 = pend_s2.pop(0)
                            f2(s2)
                            if bi == 2:
                                s2done[0] += 1
                        # tiles 0/1's gating hoisted under branch 2's conv
                        # stream as soon as the needed xc columns exist, so
                        # phase B starts with gates two tiles ahead
                        if i == 2 and s2done[0] >= 3 and gstage[0] == 0:
                            gstage[0] = 1
                            tail_lsb0 = gating_part1a(0, psT)
                            tail_gm0 = gating_part1c(
                                gating_part1b(tail_lsb0, psT))
                        elif i == 2 and s2done[0] >= 4 and gstage[0] == 1:
                            gstage[0] = 2
                            tail_g0 = gating_part2(tail_gm0, psT)
                            tail_pgbs0 = [emit_pgb_e(tail_g0, e, psT, "ps", 0)
                                          for e in range(3)]
                            tail_lsb1 = gating_part1a(1, psT)
                        elif i == 2 and s2done[0] >= 5 and gstage[0] == 2:
                            gstage[0] = 3
                            tail_g1 = gating_part2(gating_part1c(
                                gating_part1b(tail_lsb1, psT)), psT)
                        moe_math(g)
                        if g + 1 < G:
                            conv_b(g + 1)
                        while len(pend_s2) > 1:
                            bi, f2, s2 = pend_s2.pop(0)
                            f2(s2)
                            if bi == 2:
                                s2done[0] += 1
                        qk(g)
                        done += grs[g][1]
                        cov = gm["lo"] + done
                        amax = 6 if i == 2 else 12
                        while a_next < amax and \
                                _row_off(i, 4 * a_next + 3) + 96 <= cov:
                            pend_s2.append((i, attn_s2, attn_s1(a_next)))
                            a_next += 1
                    # branch tail s2's carry into the next branch's conv
                    # stream; branch 2 drains fully before the phase-B tail.
                    if i == 2:
                        while pend_s2:
                            bi, f2, s2 = pend_s2.pop(0)
                            f2(s2)
                            if bi == 2:
                                s2done[0] += 1
                        for a in range(6, 12):
                            deferred.append((attn_s1, attn_s2, a))
                # fallback if the hoist conditions never fired late enough
                if gstage[0] < 3:
                    if gstage[0] == 0:
                        tail_lsb0 = gating_part1a(0, psT)
                        tail_gm0 = gating_part1c(
                            gating_part1b(tail_lsb0, psT))
                        gstage[0] = 1
                    if gstage[0] == 1:
                        tail_g0 = gating_part2(tail_gm0, psT)
                        tail_pgbs0 = [emit_pgb_e(tail_g0, e, psT, "ps", 0)
                                      for e in range(3)]
                        tail_lsb1 = gating_part1a(1, psT)
                        gstage[0] = 2
                    tail_g1 = gating_part2(gating_part1c(
                        gating_part1b(tail_lsb1, psT)), psT)
                    gstage[0] = 3
                g0state[0] = (tail_g0, tail_pgbs0, tail_g1)
            stackX.close()

            # ---------------- Phase B: final MLP MoE + proj ---------------
            with tc.tile_pool(name="bpool", bufs=3) as bpool, \
                 tc.tile_pool(name="hpool", bufs=5) as hpool, \
                 tc.tile_pool(name="psL", bufs=3, space="PSUM") as psL, \
                 tc.tile_pool(name="psGB", bufs=1, space="PSUM") as psGB, \
                 tc.tile_pool(name="psPG", bufs=1, space="PSUM") as psPG, \
                 tc.tile_pool(name="psB", bufs=3, space="PSUM") as psB:
                w1_sb = wB["w1"]
                w2_sb = wB["w2"]
                b1_sb = wB["b1"]
                b2r_sb = wB["b2r"]
                bpr_sb = wB["bpr"]

                LA = 4
                iters = [(e, m) for e in range(3) for m in range(12)]
                gates_cur, pgbs_cur, gates_next = g0state[0]
                lsb_n2 = None
                gates_n2 = None
                dpend = []
                # deferred branch-2 attention spread over tiles 0-2
                DEF_SCHED = {(0, 4): (1, 0), (0, 14): (2, 0),
                             (0, 20): (1, 1), (0, 29): (2, 1),
                             (1, 4): (1, 2), (1, 14): (2, 2),
                             (1, 20): (1, 3), (1, 29): (2, 3),
                             (2, 4): (1, 4), (2, 14): (2, 4),
                             (2, 20): (1, 5), (2, 29): (2, 5)}
                drain_prev = [None]

                def emit_drain(final=False):
                    if drain_prev[0] is None:
                        return
                    pdp, tp0, ntp = drain_prev[0]
                    drain_prev[0] = None
                    if final:
                        # fan the last drain out across engines/queues so the
                        # tail isn't serialized behind one scalar+DMA chain
                        osb0 = bpool.tile([HD, ntp], F32, tag="osb")
                        nc.scalar.activation(
                            osb0, pdp[0], mybir.ActivationFunctionType.Identity,
                            bias=bpr_sb[:, 0:1])
                        nc.sync.dma_start(out=out_cm[0:HD, tp0:tp0 + ntp],
                                          in_=osb0)
                        osb1 = bpool.tile([HD, ntp], F32, tag="osb")
                        nc.vector.tensor_scalar_add(osb1, pdp[1],
                                                    bpr_sb[:, 1:2])
                        nc.scalar.dma_start(
                            out=out_cm[HD:2 * HD, tp0:tp0 + ntp], in_=osb1)
                        osb2 = bpool.tile([HD, ntp], F32, tag="osb")
                        nc.vector.tensor_scalar_add(osb2, pdp[2],
                                                    bpr_sb[:, 2:3])
                        nc.gpsimd.dma_start(
                            out=out_cm[2 * HD:3 * HD, tp0:tp0 + ntp], in_=osb2)
                        return
                    for mp in range(3):
                        osb = bpool.tile([HD, ntp], F32, tag="osb")
                        nc.scalar.activation(
                            osb, pdp[mp],
                            mybir.ActivationFunctionType.Identity,
                            bias=bpr_sb[:, mp:mp + 1])
                        nc.sync.dma_start(
                            out=out_cm[mp * HD:(mp + 1) * HD, tp0:tp0 + ntp],
                            in_=osb)

                segs = [(t, t * NT, NT, 0) for t in range(NTILES - 1)]
                segs.append((NTILES - 1, (NTILES - 1) * NT, 256, 0))
                segs.append((NTILES - 1, (NTILES - 1) * NT + 256, 256, 256))
                for t, t0, nt, goff in segs:
                    pd = [psL.tile([HD, nt], F32, tag="down", name=f"pd{_i}")
                          for _i in range(3)]
                    hs_l = {}
                    for k in range(36 + LA):
                        if k < 36:
                            e, m = iters[k]
                            pu = psB.tile([HD, nt], F32, tag="ps", name="pu")
                            for kc in range(3):
                                nc.tensor.matmul(
                                    pu, w1_sb[e][:, kc, m * HD:(m + 1) * HD],
                                    xc_t[kc][:, t0:t0 + nt],
                                    start=(kc == 0), stop=(kc == 2))
                            h = hpool.tile([HD, nt], BF16, tag="h")
                            nc.scalar.activation(
                                h, pu, mybir.ActivationFunctionType.Gelu,
                                bias=b1_sb[:, e, m:m + 1])
                            hs = hpool.tile([HD, nt], BF16, tag="hs")
                            nc.vector.tensor_mul(hs, h,
                                                 pgbs_cur[e][:, goff:goff + nt])
                            hs_l[k] = (e, m, hs)
                            if k == 1:
                                emit_drain()
                            if (t, k) in DEF_SCHED:
                                which, idx = DEF_SCHED[(t, k)]
                                s1f, s2f, a = deferred[idx]
                                if which == 1:
                                    dpend.append(s1f(a, psGB, psPG, "pgb"))
                                else:
                                    s2f(dpend.pop(0), psPG, "pgb")
                            if (e, m) == (0, 0) and t + 2 < NTILES:
                                lsb_n2 = gating_part1a(t + 2, psGB)
                            if (e, m) == (0, 6) and t + 2 < NTILES:
                                st_n2 = gating_part1b(lsb_n2, psGB)
                            if (e, m) == (0, 9) and t + 2 < NTILES:
                                gm_n2 = gating_part1c(st_n2)
                            if (e, m) == (1, 0) and t + 2 < NTILES:
                                gates_n2 = gating_part2(gm_n2, psGB)
                            if e == 2 and m in (7, 9, 11) and t + 1 < NTILES:
                                e_ = (m - 7) // 2
                                pb = emit_pgb_e(gates_next, e_, psPG, "pgb",
                                                t + 1)
                                if e_ == 0:
                                    pgbs_next = []
                                pgbs_next.append(pb)
                        if k >= LA:
                            e2, m2, hs2 = hs_l.pop(k - LA)
                            for mp in range(3):
                                nc.tensor.matmul(
                                    pd[mp],
                                    w2_sb[e2][:, m2, mp * HD:(mp + 1) * HD],
                                    hs2, start=(e2 == 0 and m2 == 0),
                                    stop=False)
                    for mp in range(3):
                        nc.tensor.matmul(pd[mp],
                                         b2r_sb[:, mp * HD:(mp + 1) * HD],
                                         gates_cur[:, goff:goff + nt],
                                         start=False, stop=True)
                    drain_prev[0] = (pd, t0, nt)
                    if t + 1 < NTILES:
                        gates_cur, pgbs_cur = gates_next, pgbs_next
                        gates_next = gates_n2
                emit_drain(final=True)
            stackA.close()
    nc.compile()
    return nc


def _prep_inputs(x, w_e1, b_e1, w_e2, b_e2, w_e3, b_e3, w_e4, b_e4, w_e5, b_e5,
                 w_e6, b_e6, wg1, wg2, wg3, w_qkv, w_attn_proj, b_attn_proj,
                 wg_final, w_mlp1, b_mlp1, w_mlp2, b_mlp2, w_proj, b_proj):
    f = np.float32
    shared = {}
    shared["wca"] = np.ascontiguousarray(np.stack([
        w_e1.reshape(9, HD, HD), w_e3.reshape(9, HD, HD),
        w_e5.reshape(9, HD, HD)]).transpose(0, 2, 1, 3), dtype=f).astype(BFNP)
    shared["wcb"] = np.ascontiguousarray(np.stack([
        w_e2.reshape(9, HD, HD), w_e4.reshape(9, HD, HD),
        w_e6.reshape(9, HD, HD)]).transpose(0, 2, 1, 3), dtype=f).astype(BFNP)
    shared["bca"] = np.ascontiguousarray(
        np.stack([b_e1, b_e3, b_e5], axis=1) * 0.5, dtype=f)
    shared["bcb"] = np.ascontiguousarray(
        np.stack([b_e2, b_e4, b_e6], axis=1) * 0.5, dtype=f)
    wgs = np.stack([wg1, wg2, wg3])
    shared["wgd"] = np.ascontiguousarray(
        np.repeat((wgs[:, :, 1] - wgs[:, :, 0])[:, :, None], HD, axis=2),
        dtype=f).astype(BFNP)
    eb3 = np.zeros((3, 384), f)
    for e in range(3):
        eb3[e, e * 128:(e + 1) * 128] = 1.0
    shared["eb3"] = eb3.astype(BFNP)
    wq64 = np.asarray(w_qkv[:, :, :HD], dtype=np.float64)
    wk64 = np.asarray(w_qkv[:, :, HD:256], dtype=np.float64)
    shared["wu"] = np.ascontiguousarray(
        np.einsum("icq,idq->icd", wq64, wk64), dtype=f).astype(BFNP)
    wv64 = np.asarray(w_qkv[:, :, 256:], dtype=np.float64)
    wap64 = np.asarray(w_attn_proj, dtype=np.float64)
    shared["wv"] = np.ascontiguousarray(
        np.einsum("ick,iko->ico", wv64, wap64), dtype=f).astype(BFNP)
    # fold the attention-proj bias into the MLP/gate paths (xc on device
    # is stored without it): b1' = b1 + bap @ w1; lgb = bap @ wg_final
    bap64 = np.asarray(b_attn_proj, np.float64).reshape(C)
    shared["lgb"] = np.tile(
        (bap64 @ np.asarray(wg_final, np.float64)).reshape(3, 1),
        (1, 4)).astype(f)
    shared["wgf"] = np.ascontiguousarray(
        np.tile(wg_final.reshape(3, HD, 3), (1, 1, 43))[:, :, :HD],
        dtype=f).astype(BFNP)
    shared["w1"] = np.ascontiguousarray(
        w_mlp1.reshape(3, 3, HD, 1536).transpose(0, 2, 1, 3),
        dtype=f).astype(BFNP)
    b1p = np.asarray(b_mlp1, np.float64) + \
        np.einsum("c,ecf->ef", bap64, np.asarray(w_mlp1, np.float64))
    shared["b1"] = np.ascontiguousarray(
        b1p.reshape(3, 12, HD).transpose(2, 0, 1), dtype=f)
    w2p = np.asarray(w_mlp2, dtype=np.float64) @ np.asarray(w_proj, np.float64)
    shared["w2"] = np.ascontiguousarray(
        w2p.reshape(3, 12, HD, C).transpose(0, 2, 1, 3), dtype=f).astype(BFNP)
    shared["b2r"] = np.ascontiguousarray(
        np.asarray(b_mlp2, np.float64) @ np.asarray(w_proj, np.float64),
        dtype=f).astype(BFNP)
    shared["bpr"] = np.ascontiguousarray(b_proj.reshape(3, HD).T, dtype=f)

    in_maps = []
    xf = np.asarray(x, dtype=f)
    for c in range(N_CORES):
        b, halfc = c // 2, c % 2
        r0 = halfc * R
        m = dict(shared)
        for i in range(3):
            g = GEOM[i]
            xi = xf[b, :, :, i * HD:(i + 1) * HD]  # [96, 96, 128]
            plane = np.zeros((HD, g["NR"], g["SP"]), f)
            glo = max(0, r0 - g["pad_r"])
            ghi = min(HH, r0 + R + g["pad_r"])
            plo = glo - (r0 - g["pad_r"])
            plane[:, plo:plo + (ghi - glo),
                  g["pad_c"]:g["pad_c"] + 96] = \
                xi[glo:ghi].transpose(2, 0, 1)
            buf = np.zeros((HD, g["BUF"]), f)
            buf[:, g["OFF"]:g["OFF"] + g["NR"] * g["SP"]] = \
                plane.reshape(HD, -1)
            m[f"xp{i}"] = buf.astype(BFNP)
        m["xp2"][:, GEOM[2]["OFF"] + GEOM[2]["NR"] * GEOM[2]["SP"]:] = 0
        in_maps.append(m)
    return in_maps


def kernel(**inputs):
    global _CACHED_NC
    if _CACHED_NC is None:
        _CACHED_NC = build_kernel()
    nc = _CACHED_NC
    in_maps = _prep_inputs(**{k: np.asarray(v) for k, v in inputs.items()})
    res = None
    for attempt in range(3):
        try:
            res = run_bass_kernel_spmd(nc, in_maps,
                                       core_ids=list(range(N_CORES)))
            break
        except Exception:
            if attempt == 2:
                raise
            import time
            time.sleep(2.0)
    out = np.empty((B, HH, WW, C), np.float32)
    for c in range(N_CORES):
        b, halfc = c // 2, c % 2
        slab = res.results[c]["out_cm"].reshape(C, R, 96)
        out[b, :, halfc * R:(halfc + 1) * R, :] = slab.transpose(2, 1, 0)
    return out



# revision 74
# speedup vs baseline: 1.0080x; 1.0014x over previous
"""Trainium2 Bass kernel for nn_MAMoE (conv-MoE -> row attention -> MLP-MoE).

Sharding: 8 cores = (batch b in 0..3) x (H-half in 0..1). All routing is
per-token; the reference's swapaxes(1,2) means attention row r produces
output column w=r, so each core independently computes the full pipeline
for its 48 attention rows and the host reassembles along W.

v3: all matmuls bf16 (f32 PSUM accumulate); per-branch minimal conv
padding (100/96/112 row pitch); two-stage attention emission so the
softmax chain hides under the next conv group's matmuls; software-
pipelined phase B (ups run 2 iterations ahead of downs); first tile's
gating hoisted into the phase A tail.
"""
import contextlib

import numpy as np
import ml_dtypes

import concourse.bass as bass
import concourse.bass_isa as bass_isa
import concourse.mybir as mybir
import concourse.tile as tile
from concourse import bacc
from concourse.bass_utils import run_bass_kernel_spmd
from concourse.masks import make_identity

F32 = mybir.dt.float32
BF16 = mybir.dt.bfloat16
BFNP = ml_dtypes.bfloat16

B, HH, WW, C = 4, 96, 96, 384
HD = 128
SCALE = float((HD // 3) ** -0.5)  # 42**-0.5
N_CORES = 8
R = 48            # attention rows per core
T = R * 96        # tokens per core = 4608
NT = 512          # tokens per MLP tile
NTILES = T // NT  # 9
GN = 4 * 96       # tokens per attention group = 384

# per-branch padded-plane geometry
GEOM = [
    dict(SP=100, NR=52, OFF=8, BUF=5248, pad_r=2, pad_c=2),   # 3x3 convs
    dict(SP=96, NR=64, OFF=0, BUF=6144, pad_r=8, pad_c=0),    # (9,1) convs
    dict(SP=104, NR=48, OFF=8, BUF=5120, pad_r=0, pad_c=0),   # (1,9) convs
]
for _g in GEOM:
    _g["lo"] = _g["OFF"] + _g["pad_r"] * _g["SP"]
    _g["rlen"] = 48 * _g["SP"]
GEOM[2]["rlen"] = 47 * 104 + 96

TAPS_A = [
    [(dr, ds) for dr in (-1, 0, 1) for ds in (-1, 0, 1)],
    [(dr, 0) for dr in range(-4, 5)],
    [(0, ds) for ds in range(-4, 5)],
]
TAPS_B = [
    [(dr, ds) for dr in (-2, 0, 2) for ds in (-2, 0, 2)],
    [(dr, 0) for dr in range(-8, 9, 2)],
    [(0, ds) for ds in range(-8, 9, 2)],
]


def _row_off(i, r):
    g = GEOM[i]
    return g["OFF"] + (g["pad_r"] + r) * g["SP"] + g["pad_c"]


def _groups(i):
    g = GEOM[i]
    out = []
    fo = g["lo"]
    end = g["lo"] + g["rlen"]
    while fo < end:
        out.append((fo, min(NT, end - fo)))
        fo += NT
    return out


_CACHED_NC = None


def build_kernel():
    nc = bacc.Bacc("TRN2", target_bir_lowering=False, debug=False)

    xps = [nc.dram_tensor(f"xp{i}", [HD, GEOM[i]["BUF"]], BF16,
                          kind="ExternalInput").ap() for i in range(3)]
    wca = nc.dram_tensor("wca", [3, HD, 9, HD], BF16, kind="ExternalInput").ap()
    wcb = nc.dram_tensor("wcb", [3, HD, 9, HD], BF16, kind="ExternalInput").ap()
    bca = nc.dram_tensor("bca", [HD, 3], F32, kind="ExternalInput").ap()
    bcb = nc.dram_tensor("bcb", [HD, 3], F32, kind="ExternalInput").ap()
    wgd = nc.dram_tensor("wgd", [3, HD, HD], BF16, kind="ExternalInput").ap()
    eb3 = nc.dram_tensor("eb3", [3, 384], BF16, kind="ExternalInput").ap()
    wu = nc.dram_tensor("wu", [3, HD, HD], BF16, kind="ExternalInput").ap()
    wv = nc.dram_tensor("wv", [3, HD, HD], BF16, kind="ExternalInput").ap()
    lgb = nc.dram_tensor("lgb", [3, 4], F32, kind="ExternalInput").ap()
    wgf = nc.dram_tensor("wgf", [3, HD, HD], BF16, kind="ExternalInput").ap()
    w1 = nc.dram_tensor("w1", [3, HD, 3, 1536], BF16, kind="ExternalInput").ap()
    b1 = nc.dram_tensor("b1", [HD, 3, 12], F32, kind="ExternalInput").ap()
    w2 = nc.dram_tensor("w2", [3, HD, 12, C], BF16, kind="ExternalInput").ap()
    b2r = nc.dram_tensor("b2r", [3, C], BF16, kind="ExternalInput").ap()
    bpr = nc.dram_tensor("bpr", [HD, 3], F32, kind="ExternalInput").ap()
    out_cm = nc.dram_tensor("out_cm", [C, T], F32, kind="ExternalOutput").ap()

    with tile.TileContext(nc) as tc:
        with tc.tile_pool(name="persist", bufs=1) as persist, \
             tc.tile_pool(name="wpoolB", bufs=1) as wpoolB, \
             tc.tile_pool(name="gbpool", bufs=2) as gbpool, \
             tc.tile_pool(name="gpoolB", bufs=2) as gpoolB:
            stackA = contextlib.ExitStack()
            mpool = stackA.enter_context(tc.tile_pool(name="mpool", bufs=2))
            wpoolA = stackA.enter_context(tc.tile_pool(name="wpoolA", bufs=2))
            apool = stackA.enter_context(tc.tile_pool(name="apool", bufs=4))
            anorm = stackA.enter_context(tc.tile_pool(name="anorm", bufs=2))
            stackX = contextlib.ExitStack()
            xpool = stackX.enter_context(tc.tile_pool(name="xpool", bufs=2))
            # ---- branch-0 critical DMAs first (minimize first-matmul wait);
            # spread the first transfers across 4 queues so their fixed DMA
            # latencies overlap.
            xp_sb0 = xpool.tile([HD, 6144], BF16, tag="xp")
            c1, c2 = 1024, 3072
            nc.sync.dma_start(out=xp_sb0[:, :c1], in_=xps[0][:, :c1])
            wgd_sb0 = wpoolA.tile([HD, HD], BF16, tag="wgd")
            nc.scalar.dma_start(out=wgd_sb0, in_=wgd[0])
            wca_sb0 = wpoolA.tile([HD, 9, HD], BF16, tag="wca")
            nc.gpsimd.dma_start(out=wca_sb0, in_=wca[0])
            wcb_sb0 = wpoolA.tile([HD, 9, HD], BF16, tag="wcb")
            nc.gpsimd.dma_start(out=wcb_sb0, in_=wcb[0])
            wu_sb0 = wpoolA.tile([HD, HD], BF16, tag="wu")
            nc.gpsimd.dma_start(out=wu_sb0, in_=wu[0])
            wv_sb0 = wpoolA.tile([HD, HD], BF16, tag="wv")
            nc.gpsimd.dma_start(out=wv_sb0, in_=wv[0])
            nc.gpsimd.dma_start(out=xp_sb0[:, c1:c2], in_=xps[0][:, c1:c2])
            nc.sync.dma_start(out=xp_sb0[:, c2:GEOM[0]["BUF"]],
                              in_=xps[0][:, c2:])
            bca_sb = persist.tile([HD, 3], F32)
            bcb_sb = persist.tile([HD, 3], F32)
            lgb_sb = persist.tile([3, 4], F32)
            nc.sync.dma_start(out=bca_sb, in_=bca)
            nc.sync.dma_start(out=bcb_sb, in_=bcb)
            nc.sync.dma_start(out=lgb_sb, in_=lgb)

            br0 = dict(xp=xp_sb0, wgd=wgd_sb0, wca=wca_sb0, wcb=wcb_sb0,
                       wu=wu_sb0, wv=wv_sb0)

            identb = persist.tile([HD, HD], BF16)
            make_identity(nc, identb)

            xc_t = [persist.tile([HD, T], BF16, tag=f"xc{i}", name=f"xc{i}")
                    for i in range(3)]

            wB = {}

            def emit_phaseB_weight_dmas():
                wB["b1"] = wpoolB.tile([HD, 3, 12], F32, tag="b1", name="b1s")
                nc.sync.dma_start(out=wB["b1"], in_=b1)
                wB["b2r"] = wpoolB.tile([3, C], BF16, tag="b2r", name="b2rs")
                nc.sync.dma_start(out=wB["b2r"], in_=b2r)
                wB["wgf"] = wpoolB.tile([HD, 3, HD], BF16, tag="wgf", name="wgfs")
                nc.sync.dma_start(out=wB["wgf"],
                                  in_=wgf.rearrange("a p b -> p a b"))
                wB["bpr"] = wpoolB.tile([HD, 3], F32, tag="bpr", name="bprs")
                nc.sync.dma_start(out=wB["bpr"], in_=bpr)
                wB["eb3"] = wpoolB.tile([3, 384], BF16, tag="eb3", name="eb3s")
                nc.sync.dma_start(out=wB["eb3"], in_=eb3)
                wB["w1"] = []
                wB["w2"] = []
                for e in range(3):
                    t1 = wpoolB.tile([HD, 3, 1536], BF16, tag=f"w1_{e}",
                                     name=f"w1_{e}")
                    nc.sync.dma_start(out=t1, in_=w1[e])
                    wB["w1"].append(t1)
                    t2 = wpoolB.tile([HD, 12, C], BF16, tag=f"w2_{e}",
                                     name=f"w2_{e}")
                    nc.sync.dma_start(out=t2, in_=w2[e])
                    wB["w2"].append(t2)

            def gating_part1a(t, pool):
                """logits matmul for tile t."""
                t0 = t * NT
                plg = pool.tile([HD, NT], F32, tag="ps", name="plg")
                for kc in range(3):
                    nc.tensor.matmul(plg, wB["wgf"][:, kc, :],
                                     xc_t[kc][:, t0:t0 + NT],
                                     start=(kc == 0), stop=(kc == 2))
                lsb = gpoolB.tile([3, NT], BF16, tag="lsb", name="lsb")
                nc.scalar.activation(lsb, plg[0:3, :],
                                     mybir.ActivationFunctionType.Identity,
                                     bias=lgb_sb[:, 0:1])
                return lsb

            def gating_part1b(lsb, pool):
                """token-major top-2 softmax math, first half."""
                pltf = pool.tile([HD, NT], BF16, tag="ps", name="plt")
                # 4-col stride keeps each bf16 PSUM write 4-byte aligned
                for t4 in range(4):
                    nc.tensor.transpose(pltf[:, t4 * 4:t4 * 4 + 3],
                                        lsb[:, t4 * HD:(t4 + 1) * HD],
                                        identb[:3, :3])
                lt = gpoolB.tile([HD, 12], F32, tag="lt", name="lt")
                nc.vector.tensor_copy(
                    lt.rearrange("p (g c) -> p g c", c=3),
                    pltf[:, :16].rearrange("p (g c) -> p g c", c=4)[:, :, 0:3])
                l3 = lt.rearrange("p (j e) -> p j e", e=3)
                mx = gpoolB.tile([HD, 4], F32, tag="mx", name="mx")
                nc.vector.tensor_reduce(mx, l3, axis=mybir.AxisListType.X,
                                        op=mybir.AluOpType.max)
                mn = gpoolB.tile([HD, 4], F32, tag="mn", name="mn")
                nc.vector.tensor_reduce(mn, l3, axis=mybir.AxisListType.X,
                                        op=mybir.AluOpType.min)
                sm = gpoolB.tile([HD, 4], F32, tag="sm", name="sm")
                nc.vector.tensor_reduce(sm, l3, axis=mybir.AxisListType.X,
                                        op=mybir.AluOpType.add)
                t1 = gpoolB.tile([HD, 4], F32, tag="t1", name="t1")
                nc.vector.tensor_sub(t1, sm, mx)
                mid = gpoolB.tile([HD, 4], F32, tag="mid", name="mid")
                nc.vector.tensor_sub(mid, t1, mn)
                dm = gpoolB.tile([HD, 4], F32, tag="dm", name="dm")
                nc.vector.tensor_sub(dm, mx, mid)
                th = gpoolB.tile([HD, 4], F32, tag="th", name="th")
                nc.scalar.activation(th, dm,
                                     mybir.ActivationFunctionType.Tanh,
                                     scale=0.5)
                return (lt, mx, mn, th)

            def gating_part1c(st):
                """token-major top-2 softmax math, second half."""
                lt, mx, mn, th = st
                gmx = gpoolB.tile([HD, 4], F32, tag="gmx", name="gmx")
                nc.vector.tensor_scalar(gmx, th, 0.5, 0.5,
                                        op0=mybir.AluOpType.mult,
                                        op1=mybir.AluOpType.add)
                eqx = gpoolB.tile([HD, 12], F32, tag="eqx", name="eqx")
                eqn = gpoolB.tile([HD, 12], F32, tag="eqn", name="eqn")
                for t4 in range(4):
                    sl = slice(t4 * 3, (t4 + 1) * 3)
                    nc.vector.tensor_scalar(eqx[:, sl], lt[:, sl],
                                            mx[:, t4:t4 + 1], None,
                                            op0=mybir.AluOpType.is_equal)
                    nc.vector.tensor_scalar(eqn[:, sl], lt[:, sl],
                                            mn[:, t4:t4 + 1], None,
                                            op0=mybir.AluOpType.is_equal)
                # u = 1 - eqx - eqn (mid indicator); g = gmx*(eqx-u) + u
                s1 = gpoolB.tile([HD, 12], F32, tag="s1", name="s1")
                nc.vector.tensor_add(s1, eqx, eqn)
                u = gpoolB.tile([HD, 12], F32, tag="u", name="u")
                nc.vector.tensor_scalar(u, s1, -1.0, 1.0,
                                        op0=mybir.AluOpType.mult,
                                        op1=mybir.AluOpType.add)
                d0 = gpoolB.tile([HD, 12], F32, tag="d0", name="d0")
                nc.vector.tensor_sub(d0, eqx, u)
                p0 = gpoolB.tile([HD, 12], F32, tag="p0", name="p0")
                for t4 in range(4):
                    sl = slice(t4 * 3, (t4 + 1) * 3)
                    nc.vector.tensor_scalar_mul(p0[:, sl], d0[:, sl],
                                                gmx[:, t4:t4 + 1])
                gm2 = gpoolB.tile([HD, 12], BF16, tag="gm", name="gm")
                nc.vector.tensor_add(gm2, p0, u)
                return gm2

            def gating_part2(gm2, pool):
                """expert-major gates [3, NT] from token-major gm."""
                pgtf = pool.tile([HD, NT], BF16, tag="ps", name="pgt")
                pgt = pgtf[0:3, :]
                for t4 in range(4):
                    nc.tensor.transpose(pgt[:, t4 * HD:(t4 + 1) * HD],
                                        gm2[:, t4 * 3:(t4 + 1) * 3],
                                        identb)
                gates_r = gpoolB.tile([3, NT], BF16, tag="gates",
                                      name="gates_r", bufs=3)
                nc.scalar.copy(gates_r, pgt)
                return gates_r

            def emit_pgb_e(gates_r, e, pspool, tag, t):
                """gate row e broadcast to 128 partitions via one-hot MM."""
                pgb = pspool.tile([HD, NT], F32, tag=tag, name="pgbp")
                nc.tensor.matmul(pgb, wB["eb3"][:, e * HD:(e + 1) * HD],
                                 gates_r, start=True, stop=True)
                pb = gbpool.tile([HD, NT], BF16, tag=f"pgb{e}",
                                 name=f"pgb{e}_{t}")
                # two half-copies interleave with the gelu stream on the
                # scalar queue instead of one 700ns burst
                nc.scalar.copy(pb[:, :NT // 2], pgb[:, :NT // 2])
                nc.scalar.copy(pb[:, NT // 2:], pgb[:, NT // 2:])
                return pb

            g0state = [None]

            # ------------ Phase A: conv MoE + attention per branch --------
            deferred = []
            pend_s2 = []
            s2done = [0]
            gstage = [0]
            with tc.tile_pool(name="gpool", bufs=2) as gpool, \
                 tc.tile_pool(name="psC", bufs=4, space="PSUM") as psC, \
                 tc.tile_pool(name="psT", bufs=4, space="PSUM") as psT:
                for i in range(3):
                    gm = GEOM[i]
                    grs = _groups(i)
                    G = len(grs)
                    if i == 0:
                        xp_sb = br0["xp"]
                        wgd_sb, wca_sb, wcb_sb = \
                            br0["wgd"], br0["wca"], br0["wcb"]
                        wu_sb, wv_sb = br0["wu"], br0["wv"]
                    else:
                        xp_sb = xpool.tile([HD, 6144], BF16, tag="xp")
                        nc.gpsimd.dma_start(out=xp_sb[:, :c1],
                                            in_=xps[i][:, :c1])
                        wgd_sb = wpoolA.tile([HD, HD], BF16, tag="wgd")
                        nc.gpsimd.dma_start(out=wgd_sb, in_=wgd[i])
                        wca_sb = wpoolA.tile([HD, 9, HD], BF16, tag="wca")
                        nc.gpsimd.dma_start(out=wca_sb, in_=wca[i])
                        wcb_sb = wpoolA.tile([HD, 9, HD], BF16, tag="wcb")
                        nc.gpsimd.dma_start(out=wcb_sb, in_=wcb[i])
                        wu_sb = wpoolA.tile([HD, HD], BF16, tag="wu")
                        nc.gpsimd.dma_start(out=wu_sb, in_=wu[i])
                        wv_sb = wpoolA.tile([HD, HD], BF16, tag="wv")
                        nc.gpsimd.dma_start(out=wv_sb, in_=wv[i])
                        nc.gpsimd.dma_start(out=xp_sb[:, c1:c2],
                                            in_=xps[i][:, c1:c2])
                        nc.sync.dma_start(out=xp_sb[:, c2:gm["BUF"]],
                                          in_=xps[i][:, c2:])
                    if i == 1:
                        emit_phaseB_weight_dmas()

                    # max used extent: branch 1 (lo=768, rlen=4608 -> 5376)
                    moe_buf = mpool.tile([HD, 5376], BF16, tag="moe")
                    u_buf = mpool.tile([HD, 5376], BF16, tag="u")

                    st = {}

                    def conv_a(g):
                        fo, n = grs[g]
                        plg = psC.tile([HD, NT], F32, tag="ps", name="plg")
                        nc.tensor.matmul(plg[:, :n], wgd_sb,
                                         xp_sb[:, fo:fo + n],
                                         start=True, stop=True)
                        ex = gpool.tile([HD, NT], BF16, tag="ex")
                        nc.scalar.activation(ex[:, :n], plg[:, :n],
                                             mybir.ActivationFunctionType.Tanh,
                                             scale=-0.5)
                        pa = psC.tile([HD, NT], F32, tag="ps", name="pa")
                        for ti, (dr, ds) in enumerate(TAPS_A[i]):
                            o = dr * gm["SP"] + ds
                            nc.tensor.matmul(pa[:, :n], wca_sb[:, ti, :],
                                             xp_sb[:, fo + o: fo + o + n],
                                             start=(ti == 0), stop=(ti == 8))
                        st[g] = (ex, pa)

                    def conv_b(g):
                        fo, n = grs[g]
                        pb = psC.tile([HD, NT], F32, tag="ps", name="pb")
                        for ti, (dr, ds) in enumerate(TAPS_B[i]):
                            o = dr * gm["SP"] + ds
                            nc.tensor.matmul(pb[:, :n], wcb_sb[:, ti, :],
                                             xp_sb[:, fo + o: fo + o + n],
                                             start=(ti == 0), stop=(ti == 8))
                        st[g] = st[g] + (pb,)

                    def moe_math(g):
                        fo, n = grs[g]
                        ex, pa, pb = st.pop(g)
                        ca = gpool.tile([HD, NT], BF16, tag="ca")
                        nc.scalar.activation(ca[:, :n], pa[:, :n],
                                             mybir.ActivationFunctionType.Identity,
                                             bias=bca_sb[:, i:i + 1], scale=0.5)
                        cb = gpool.tile([HD, NT], BF16, tag="cb")
                        nc.scalar.activation(cb[:, :n], pb[:, :n],
                                             mybir.ActivationFunctionType.Identity,
                                             bias=bcb_sb[:, i:i + 1], scale=0.5)
                        dd = gpool.tile([HD, NT], BF16, tag="dd")
                        nc.vector.tensor_sub(dd[:, :n], ca[:, :n], cb[:, :n])
                        d2 = gpool.tile([HD, NT], BF16, tag="d2")
                        nc.vector.tensor_mul(d2[:, :n], dd[:, :n], ex[:, :n])
                        ss = gpool.tile([HD, NT], BF16, tag="ss")
                        nc.vector.tensor_add(ss[:, :n], ca[:, :n], cb[:, :n])
                        nc.vector.tensor_add(moe_buf[:, fo:fo + n],
                                             ss[:, :n], d2[:, :n])

                    def qk(g):
                        fo, n = grs[g]
                        pq = psT.tile([HD, NT], F32, tag="ps", name="pq")
                        nc.tensor.matmul(pq[:, :n], wu_sb,
                                         moe_buf[:, fo:fo + n],
                                         start=True, stop=True)
                        nc.scalar.copy(u_buf[:, fo:fo + n], pq[:, :n])

                    def attn_s1(a, pspool=psT, pscpool=None, psctag="ps"):
                        if pscpool is None:
                            pscpool = pspool
                        offs = [_row_off(i, 4 * a + j) for j in range(4)]
                        pvt = pspool.tile([96, 4 * HD], F32, tag="ps", name="pvt")
                        for j in range(4):
                            nc.tensor.matmul(pvt[:, j * HD:(j + 1) * HD],
                                             moe_buf[:, offs[j]:offs[j] + 96],
                                             wv_sb, start=True, stop=True)
                        vt = apool.tile([96, 4 * HD], BF16, tag="vt")
                        nc.vector.tensor_copy(vt, pvt)
                        psc = pscpool.tile([96, GN], F32, tag=psctag,
                                           name="psc")
                        for j in range(4):
                            nc.tensor.matmul(psc[:, j * 96:(j + 1) * 96],
                                             u_buf[:, offs[j]:offs[j] + 96],
                                             moe_buf[:, offs[j]:offs[j] + 96],
                                             start=True, stop=True)
                        probs = apool.tile([96, GN], BF16, tag="probs")
                        nc.scalar.activation(probs, psc,
                                             mybir.ActivationFunctionType.Exp,
                                             scale=SCALE)
                        zsum = apool.tile([96, 4], F32, tag="zsum")
                        nc.vector.tensor_reduce(
                            zsum, probs.rearrange("p (j q) -> p j q", q=96),
                            axis=mybir.AxisListType.X, op=mybir.AluOpType.add)
                        rec = apool.tile([96, 4], F32, tag="rec")
                        nc.vector.reciprocal(rec, zsum)
                        pn = apool.tile([96, GN], BF16, tag="pn")
                        for j in range(4):
                            nc.vector.tensor_scalar_mul(
                                pn[:, j * 96:(j + 1) * 96],
                                probs[:, j * 96:(j + 1) * 96],
                                rec[:, j:j + 1])
                        return (a, vt, pn)

                    def attn_s2(s, pspool=psT, ptag="ps", i=i):
                        a, vt, pn = s
                        ppt = pspool.tile([96, GN], BF16, tag=ptag, name="ppt")
                        for j in range(4):
                            nc.tensor.transpose(ppt[:, j * 96:(j + 1) * 96],
                                                pn[:, j * 96:(j + 1) * 96],
                                                identb[:96, :96])
                        pt = apool.tile([96, GN], BF16, tag="pt")
                        nc.vector.tensor_copy(pt, ppt)
                        po = pspool.tile([HD, GN], F32, tag=ptag, name="po")
                        for j in range(4):
                            nc.tensor.matmul(po[:, j * 96:(j + 1) * 96],
                                             vt[:, j * HD:(j + 1) * HD],
                                             pt[:, j * 96:(j + 1) * 96],
                                             start=True, stop=True)
                        nc.scalar.copy(xc_t[i][:, a * GN:(a + 1) * GN], po)

                    conv_a(0)
                    conv_b(0)
                    a_next = 0
                    done = 0
                    for g in range(G):
                        if g + 1 < G:
                            conv_a(g + 1)
                        # drain at most one pending s2 here; the rest after
                        # conv_b so the softmax vector chain has more slack
                        if len(pend_s2) > 1:
                            bi, f2, s2 = pend_s2.pop(0)
                            f2(s2)
                            if bi == 2:
                                s2done[0] += 1
                        # tiles 0/1's gating hoisted under branch 2's conv
                        # stream as soon as the needed xc columns exist, so
                        # phase B starts with gates two tiles ahead
                        if i == 2 and s2done[0] >= 3 and gstage[0] == 0:
                            gstage[0] = 1
                            tail_lsb0 = gating_part1a(0, psT)
                            tail_gm0 = gating_part1c(
                                gating_part1b(tail_lsb0, psT))
                        elif i == 2 and s2done[0] >= 4 and gstage[0] == 1:
                            gstage[0] = 2
                            tail_g0 = gating_part2(tail_gm0, psT)
                            tail_pgbs0 = [emit_pgb_e(tail_g0, e, psT, "ps", 0)
                                          for e in range(3)]
                            tail_lsb1 = gating_part1a(1, psT)
                        elif i == 2 and s2done[0] >= 5 and gstage[0] == 2:
                            gstage[0] = 3
                            tail_g1 = gating_part2(gating_part1c(
                                gating_part1b(tail_lsb1, psT)), psT)
                        moe_math(g)
                        if g + 1 < G:
                            conv_b(g + 1)
                        while len(pend_s2) > 1:
                            bi, f2, s2 = pend_s2.pop(0)
                            f2(s2)
                            if bi == 2:
                                s2done[0] += 1
                        qk(g)
                        done += grs[g][1]
                        cov = gm["lo"] + done
                        amax = 6 if i == 2 else 12
                        while a_next < amax and \
                                _row_off(i, 4 * a_next + 3) + 96 <= cov:
                            pend_s2.append((i, attn_s2, attn_s1(a_next)))
                            a_next += 1
                    # branch tail s2's carry into the next branch's conv
                    # stream; branch 2 drains fully before the phase-B tail.
                    if i == 2:
                        while pend_s2:
                            bi, f2, s2 = pend_s2.pop(0)
                            f2(s2)
                            if bi == 2:
                                s2done[0] += 1
                        for a in range(6, 12):
                            deferred.append((attn_s1, attn_s2, a))
                # fallback if the hoist conditions never fired late enough
                if gstage[0] < 3:
                    if gstage[0] == 0:
                        tail_lsb0 = gating_part1a(0, psT)
                        tail_gm0 = gating_part1c(
                            gating_part1b(tail_lsb0, psT))
                        gstage[0] = 1
                    if gstage[0] == 1:
                        tail_g0 = gating_part2(tail_gm0, psT)
                        tail_pgbs0 = [emit_pgb_e(tail_g0, e, psT, "ps", 0)
                                      for e in range(3)]
                        tail_lsb1 = gating_part1a(1, psT)
                        gstage[0] = 2
                    tail_g1 = gating_part2(gating_part1c(
                        gating_part1b(tail_lsb1, psT)), psT)
                    gstage[0] = 3
                g0state[0] = (tail_g0, tail_pgbs0, tail_g1)
            stackX.close()

            # ---------------- Phase B: final MLP MoE + proj ---------------
            with tc.tile_pool(name="bpool", bufs=3) as bpool, \
                 tc.tile_pool(name="hpool", bufs=5) as hpool, \
                 tc.tile_pool(name="psL", bufs=3, space="PSUM") as psL, \
                 tc.tile_pool(name="psGB", bufs=1, space="PSUM") as psGB, \
                 tc.tile_pool(name="psPG", bufs=1, space="PSUM") as psPG, \
                 tc.tile_pool(name="psB", bufs=3, space="PSUM") as psB:
                w1_sb = wB["w1"]
                w2_sb = wB["w2"]
                b1_sb = wB["b1"]
                b2r_sb = wB["b2r"]
                bpr_sb = wB["bpr"]

                LA = 4
                iters = [(e, m) for e in range(3) for m in range(12)]
                gates_cur, pgbs_cur, gates_next = g0state[0]
                lsb_n2 = None
                gates_n2 = None
                dpend = []
                # deferred branch-2 attention spread over tiles 0-2
                DEF_SCHED = {(0, 4): (1, 0), (0, 14): (2, 0),
                             (0, 20): (1, 1), (0, 29): (2, 1),
                             (1, 4): (1, 2), (1, 14): (2, 2),
                             (1, 20): (1, 3), (1, 29): (2, 3),
                             (2, 4): (1, 4), (2, 14): (2, 4),
                             (2, 20): (1, 5), (2, 29): (2, 5)}
                drain_prev = [None]

                def emit_drain(final=False):
                    if drain_prev[0] is None:
                        return
                    pdp, tp0 = drain_prev[0]
                    drain_prev[0] = None
                    if final:
                        # fan the last drain out across engines/queues so the
                        # tail isn't serialized behind one scalar+DMA chain
                        osb0 = bpool.tile([HD, NT], F32, tag="osb")
                        nc.scalar.activation(
                            osb0, pdp[0], mybir.ActivationFunctionType.Identity,
                            bias=bpr_sb[:, 0:1])
                        nc.sync.dma_start(out=out_cm[0:HD, tp0:tp0 + NT],
                                          in_=osb0)
                        osb1 = bpool.tile([HD, NT], F32, tag="osb")
                        nc.vector.tensor_scalar_add(osb1, pdp[1],
                                                    bpr_sb[:, 1:2])
                        nc.scalar.dma_start(
                            out=out_cm[HD:2 * HD, tp0:tp0 + NT], in_=osb1)
                        osb2 = bpool.tile([HD, NT], F32, tag="osb")
                        nc.vector.tensor_scalar_add(osb2, pdp[2],
                                                    bpr_sb[:, 2:3])
                        nc.gpsimd.dma_start(
                            out=out_cm[2 * HD:3 * HD, tp0:tp0 + NT], in_=osb2)
                        return
                    for mp in range(3):
                        osb = bpool.tile([HD, NT], F32, tag="osb")
                        nc.scalar.activation(
                            osb, pdp[mp],
                            mybir.ActivationFunctionType.Identity,
                            bias=bpr_sb[:, mp:mp + 1])
                        nc.sync.dma_start(
                            out=out_cm[mp * HD:(mp + 1) * HD, tp0:tp0 + NT],
                            in_=osb)

                for t in range(NTILES):
                    t0 = t * NT
                    pd = [psL.tile([HD, NT], F32, tag="down", name=f"pd{_i}")
                          for _i in range(3)]
                    hs_l = {}
                    for k in range(36 + LA):
                        if k < 36:
                            e, m = iters[k]
                            pu = psB.tile([HD, NT], F32, tag="ps", name="pu")
                            for kc in range(3):
                                nc.tensor.matmul(
                                    pu, w1_sb[e][:, kc, m * HD:(m + 1) * HD],
                                    xc_t[kc][:, t0:t0 + NT],
                                    start=(kc == 0), stop=(kc == 2))
                            h = hpool.tile([HD, NT], BF16, tag="h")
                            nc.scalar.activation(
                                h, pu, mybir.ActivationFunctionType.Gelu,
                                bias=b1_sb[:, e, m:m + 1])
                            hs = hpool.tile([HD, NT], BF16, tag="hs")
                            nc.vector.tensor_mul(hs, h, pgbs_cur[e])
                            hs_l[k] = (e, m, hs)
                            if k == 1:
                                emit_drain()
                            if (t, k) in DEF_SCHED:
                                which, idx = DEF_SCHED[(t, k)]
                                s1f, s2f, a = deferred[idx]
                                if which == 1:
                                    dpend.append(s1f(a, psGB, psPG, "pgb"))
                                else:
                                    s2f(dpend.pop(0), psPG, "pgb")
                            if (e, m) == (0, 0) and t + 2 < NTILES:
                                lsb_n2 = gating_part1a(t + 2, psGB)
                            if (e, m) == (0, 6) and t + 2 < NTILES:
                                st_n2 = gating_part1b(lsb_n2, psGB)
                            if (e, m) == (0, 9) and t + 2 < NTILES:
                                gm_n2 = gating_part1c(st_n2)
                            if (e, m) == (1, 0) and t + 2 < NTILES:
                                gates_n2 = gating_part2(gm_n2, psGB)
                            if e == 2 and m in (7, 9, 11) and t + 1 < NTILES:
                                e_ = (m - 7) // 2
                                pb = emit_pgb_e(gates_next, e_, psPG, "pgb",
                                                t + 1)
                                if e_ == 0:
                                    pgbs_next = []
                                pgbs_next.append(pb)
                        if k >= LA:
                            e2, m2, hs2 = hs_l.pop(k - LA)
                            for mp in range(3):
                                nc.tensor.matmul(
                                    pd[mp],
                                    w2_sb[e2][:, m2, mp * HD:(mp + 1) * HD],
                                    hs2, start=(e2 == 0 and m2 == 0),
                                    stop=False)
                    for mp in range(3):
                        nc.tensor.matmul(pd[mp],
                                         b2r_sb[:, mp * HD:(mp + 1) * HD],
                                         gates_cur, start=False, stop=True)
                    drain_prev[0] = (pd, t0)
                    if t + 1 < NTILES:
                        gates_cur, pgbs_cur = gates_next, pgbs_next
                        gates_next = gates_n2
                emit_drain(final=True)
            stackA.close()
    nc.compile()
    return nc


def _prep_inputs(x, w_e1, b_e1, w_e2, b_e2, w_e3, b_e3, w_e4, b_e4, w_e5, b_e5,
                 w_e6, b_e6, wg1, wg2, wg3, w_qkv, w_attn_proj, b_attn_proj,
                 wg_final, w_mlp1, b_mlp1, w_mlp2, b_mlp2, w_proj, b_proj):
    f = np.float32
    shared = {}
    shared["wca"] = np.ascontiguousarray(np.stack([
        w_e1.reshape(9, HD, HD), w_e3.reshape(9, HD, HD),
        w_e5.reshape(9, HD, HD)]).transpose(0, 2, 1, 3), dtype=f).astype(BFNP)
    shared["wcb"] = np.ascontiguousarray(np.stack([
        w_e2.reshape(9, HD, HD), w_e4.reshape(9, HD, HD),
        w_e6.reshape(9, HD, HD)]).transpose(0, 2, 1, 3), dtype=f).astype(BFNP)
    shared["bca"] = np.ascontiguousarray(
        np.stack([b_e1, b_e3, b_e5], axis=1) * 0.5, dtype=f)
    shared["bcb"] = np.ascontiguousarray(
        np.stack([b_e2, b_e4, b_e6], axis=1) * 0.5, dtype=f)
    wgs = np.stack([wg1, wg2, wg3])
    shared["wgd"] = np.ascontiguousarray(
        np.repeat((wgs[:, :, 1] - wgs[:, :, 0])[:, :, None], HD, axis=2),
        dtype=f).astype(BFNP)
    eb3 = np.zeros((3, 384), f)
    for e in range(3):
        eb3[e, e * 128:(e + 1) * 128] = 1.0
    shared["eb3"] = eb3.astype(BFNP)
    wq64 = np.asarray(w_qkv[:, :, :HD], dtype=np.float64)
    wk64 = np.asarray(w_qkv[:, :, HD:256], dtype=np.float64)
    shared["wu"] = np.ascontiguousarray(
        np.einsum("icq,idq->icd", wq64, wk64), dtype=f).astype(BFNP)
    wv64 = np.asarray(w_qkv[:, :, 256:], dtype=np.float64)
    wap64 = np.asarray(w_attn_proj, dtype=np.float64)
    shared["wv"] = np.ascontiguousarray(
        np.einsum("ick,iko->ico", wv64, wap64), dtype=f).astype(BFNP)
    # fold the attention-proj bias into the MLP/gate paths (xc on device
    # is stored without it): b1' = b1 + bap @ w1; lgb = bap @ wg_final
    bap64 = np.asarray(b_attn_proj, np.float64).reshape(C)
    shared["lgb"] = np.tile(
        (bap64 @ np.asarray(wg_final, np.float64)).reshape(3, 1),
        (1, 4)).astype(f)
    shared["wgf"] = np.ascontiguousarray(
        np.tile(wg_final.reshape(3, HD, 3), (1, 1, 43))[:, :, :HD],
        dtype=f).astype(BFNP)
    shared["w1"] = np.ascontiguousarray(
        w_mlp1.reshape(3, 3, HD, 1536).transpose(0, 2, 1, 3),
        dtype=f).astype(BFNP)
    b1p = np.asarray(b_mlp1, np.float64) + \
        np.einsum("c,ecf->ef", bap64, np.asarray(w_mlp1, np.float64))
    shared["b1"] = np.ascontiguousarray(
        b1p.reshape(3, 12, HD).transpose(2, 0, 1), dtype=f)
    w2p = np.asarray(w_mlp2, dtype=np.float64) @ np.asarray(w_proj, np.float64)
    shared["w2"] = np.ascontiguousarray(
        w2p.reshape(3, 12, HD, C).transpose(0, 2, 1, 3), dtype=f).astype(BFNP)
    shared["b2r"] = np.ascontiguousarray(
        np.asarray(b_mlp2, np.float64) @ np.asarray(w_proj, np.float64),
        dtype=f).astype(BFNP)
    shared["bpr"] = np.ascontiguousarray(b_proj.reshape(3, HD).T, dtype=f)

    in_maps = []
    xf = np.asarray(x, dtype=f)
    for c in range(N_CORES):
        b, halfc = c // 2, c % 2
        r0 = halfc * R
        m = dict(shared)
        for i in range(3):
            g = GEOM[i]
            xi = xf[b, :, :, i * HD:(i + 1) * HD]  # [96, 96, 128]
            plane = np.zeros((HD, g["NR"], g["SP"]), f)
            glo = max(0, r0 - g["pad_r"])
            ghi = min(HH, r0 + R + g["pad_r"])
            plo = glo - (r0 - g["pad_r"])
            plane[:, plo:plo + (ghi - glo),
                  g["pad_c"]:g["pad_c"] + 96] = \
                xi[glo:ghi].transpose(2, 0, 1)
            buf = np.zeros((HD, g["BUF"]), f)
            buf[:, g["OFF"]:g["OFF"] + g["NR"] * g["SP"]] = \
                plane.reshape(HD, -1)
            m[f"xp{i}"] = buf.astype(BFNP)
        m["xp2"][:, GEOM[2]["OFF"] + GEOM[2]["NR"] * GEOM[2]["SP"]:] = 0
        in_maps.append(m)
    return in_maps


def kernel(**inputs):
    global _CACHED_NC
    if _CACHED_NC is None:
        _CACHED_NC = build_kernel()
    nc = _CACHED_NC
    in_maps = _prep_inputs(**{k: np.asarray(v) for k, v in inputs.items()})
    res = None
    for attempt in range(3):
        try:
            res = run_bass_kernel_spmd(nc, in_maps,
                                       core_ids=list(range(N_CORES)))
            break
        except Exception:
            if attempt == 2:
                raise
            import time
            time.sleep(2.0)
    out = np.empty((B, HH, WW, C), np.float32)
    for c in range(N_CORES):
        b, halfc = c // 2, c % 2
        slab = res.results[c]["out_cm"].reshape(C, R, 96)
        out[b, :, halfc * R:(halfc + 1) * R, :] = slab.transpose(2, 1, 0)
    return out

